# revision 6
# baseline (speedup 1.0000x reference)
"""Bass/Trainium2 kernel for nn_DilatedResBlock (RandLA-Net style block).

Sharding: batch B=2 x 4-way split of N=8192 points -> 8 cores.
Each core: KNN top-17 for its 2048 rows (full-batch candidate scan via PE
matmul metric + VectorE max8/match_replace rounds), geometry encoding via
algebraic fold (per-point A-tables + per-row S-matmul), two attentive
pooling stages with per-neighbor record gathers (SWDGE dma_gather), one
AllGather of stage-0 outputs across the 4 cores of a batch.
"""

import numpy as np

import concourse.bass as bass
import concourse.mybir as mybir
from concourse import bacc
from concourse.bass_utils import run_bass_kernel_spmd
from concourse.tile import TileContext

F32 = mybir.dt.float32
U16 = mybir.dt.uint16
I16 = mybir.dt.int16

B, DIMS, K1 = 2, 2, 17
IN_UNITS, UNITS = 64, 128
EPS = 1e-8
NEG = -3.0e38

AF = mybir.ActivationFunctionType


def host_weights(w, N):
    """Host-side rearrangements of the given weights (no data-dependent work)."""
    f = np.float32
    Wl0, bl0 = w["Wl0"], w["bl0"]
    Wl1, bl1 = w["Wl1"], w["bl1"]
    # A-fold: r0_pre[n,j] = [x_j,y_j,r_j,1] @ w_a + S[n]
    def afold(Wl, bl):
        xco = Wl[0] - Wl[2::3].sum(0)
        yco = Wl[1] - Wl[3::3].sum(0)
        rco = Wl[4::3].sum(0)
        return np.stack([xco, yco, rco, bl]).astype(f)

    w_a0 = afold(Wl0, bl0)          # [4, 32]
    w_a1 = afold(Wl1, bl1)          # [4, 64]

    # S + stats matmul weights: coords5 channel (k, j), j in {x,y,x2,y2,xy}
    w_ss = np.zeros((5 * K1, 32 + 5 + 64), f)
    for k in range(K1):
        w_ss[5 * k + 0, 0:32] = Wl0[2 + 3 * k]
        w_ss[5 * k + 1, 0:32] = Wl0[3 + 3 * k]
        w_ss[5 * k + 0, 32] = 1.0
        w_ss[5 * k + 1, 33] = 1.0
        w_ss[5 * k + 2, 34] = 1.0
        w_ss[5 * k + 3, 35] = 1.0
        w_ss[5 * k + 4, 36] = 1.0
        w_ss[5 * k + 0, 37:101] = Wl1[2 + 3 * k]
        w_ss[5 * k + 1, 37:101] = Wl1[3 + 3 * k]

    vs = lambda W, b: np.vstack([W, b[None, :]]).astype(f)
    return {
        "w_a0": w_a0, "w_a1": w_a1, "w_ss": w_ss.astype(f),
        "w_mlp0": vs(w["W0"], w["b0"]),          # [65, 32]
        "w_res": vs(w["W_res"], w["b_res"]),     # [65, 128]
        "w_p0": vs(w["Ws0"][0:32], w["bs0"]),    # [33, 64]
        "w_sr0": w["Ws0"][32:64].astype(f),      # [32, 64]
        "w_p1": vs(w["Ws1"][0:64], w["bs1"]),    # [65, 128]
        "w_sr1": w["Ws1"][64:128].astype(f),     # [64, 128]
        "w_f0": vs(w["Wf0"], w["bf0"]),          # [67, 64]
        "w_f1a": w["Wf1"][0:128].astype(f),      # [128, 128]
        "w_f1b": vs(w["Wf1"][128:130], w["bf1"]),# [3, 128]
        "w_out": w["W1"].astype(f),              # [128, 128]
        "b_out": w["b1"].reshape(128, 1).astype(f),
        "ident": np.eye(128, dtype=f),
        "ones_n": np.ones((1, min(N, 2048)), f),
    }


WNAMES = ["w_a0", "w_a1", "w_ss", "w_mlp0", "w_res", "w_p0", "w_sr0",
          "w_p1", "w_sr1", "w_f0", "w_f1a", "w_f1b", "w_out", "b_out",
          "ident", "ones_n"]
WSHAPES = None  # filled in build


def build_module(N, CPB, n_cores, no_cc=False, skip_topk=False,
                 gather_mode="swdge"):
    """Build the SPMD Bass module. R = N // CPB own rows per core."""
    R = N // CPB
    T = R // 128      # own row tiles
    PT = N // 128     # full-batch point tiles
    REC0 = 192
    REC1 = 256
    S17 = 136         # wrapped idx cols per tile (17*8)

    nc = bacc.Bacc("TRN2", target_bir_lowering=False, debug=False,
                   num_devices=n_cores)

    # --- I/O ---
    pc_b = nc.dram_tensor("pc_b", [N, 2], F32, kind="ExternalInput")
    feats_b = nc.dram_tensor("feats_b", [N, 64], F32, kind="ExternalInput")
    pc_own = nc.dram_tensor("pc_own", [R, 2], F32, kind="ExternalInput")
    feats_own = nc.dram_tensor("feats_own", [R, 64], F32, kind="ExternalInput")
    wt = {}
    shapes = {"w_a0": [4, 32], "w_a1": [4, 64], "w_ss": [85, 101],
              "w_mlp0": [65, 32], "w_res": [65, 128], "w_p0": [33, 64],
              "w_sr0": [32, 64], "w_p1": [65, 128], "w_sr1": [64, 128],
              "w_f0": [67, 64], "w_f1a": [128, 128], "w_f1b": [3, 128],
              "w_out": [128, 128], "b_out": [128, 1],
              "ident": [128, 128], "ones_n": [1, min(N, 2048)]}
    for nm in WNAMES:
        wt[nm] = nc.dram_tensor(nm, shapes[nm], F32, kind="ExternalInput")
    out = nc.dram_tensor("out", [R, 128], F32, kind="ExternalOutput")

    table0 = nc.dram_tensor("table0", [N, REC0], F32)
    table1_own = nc.dram_tensor("table1_own", [R, REC1], F32)
    table1 = nc.dram_tensor("table1", [N, REC1], F32)

    groups = [[g * CPB + j for j in range(CPB)] for g in range(n_cores // CPB)]

    with TileContext(nc) as tc:
        # ---- persistent pool (lives whole kernel) ----
        pp = tc.alloc_tile_pool(name="persist", bufs=1)
        wsb = {nm: pp.tile(shapes[nm], F32, tag=nm, name=nm) for nm in WNAMES}
        for nm in WNAMES:
            nc.sync.dma_start(out=wsb[nm], in_=wt[nm].ap())
        ident = wsb["ident"]

        def tr(out_ap, in_ap):
            k = in_ap.shape[0]
            nc.tensor.transpose(out_ap, in_ap, ident[0:k, 0:k])

        rhs3 = pp.tile([3, N], F32, tag="rhs3")
        pcT_own = pp.tile([2, R], F32, tag="pcT_own")
        y_resT = pp.tile([128, R], F32, tag="y_resT")
        S1_keep = pp.tile([128, T * 64], F32, tag="S1_keep")
        ggf_keep = pp.tile([128, T * 2], F32, tag="ggf_keep")
        x1T = pp.tile([65, R], F32, tag="x1T")
        idxs_u16 = pp.tile([128, T * 24], U16, tag="idxs_u16")
        widx = pp.tile([128, T * S17], I16, tag="widx")

        # ================= PHASE A: tables =================
        OC = min(N, 2048)

        def ones_dma(dst_ap, length):
            for c in range(0, length, OC):
                ce = min(c + OC, length)
                nc.sync.dma_start(out=dst_ap[:, c:ce],
                                  in_=wt["ones_n"].ap()[:, 0:ce - c])

        # ---- A-geom scope ----
        with tc.tile_pool(name="pag", bufs=1) as pa, \
             tc.tile_pool(name="pag_ch", bufs=2) as pch, \
             tc.tile_pool(name="pa_ps", bufs=3, space="PSUM") as pa_ps:
            pc_rm = pa.tile([128, PT * 2], F32, tag="pc_rm")
            nc.sync.dma_start(out=pc_rm.rearrange("p (t c) -> p t c", c=2),
                              in_=pc_b.ap().rearrange("(t p) c -> p t c", p=128))
            pco_rm = pa.tile([128, T * 2], F32, tag="pco_rm")
            nc.sync.dma_start(out=pco_rm.rearrange("p (t c) -> p t c", c=2),
                              in_=pc_own.ap().rearrange("(t p) c -> p t c", p=128))

            # cA = [x; y; r; 1] channel-major, full batch
            cA = pa.tile([4, N], F32, tag="cA")
            for t in range(PT):
                ps = pa_ps.tile([2, 128], F32, tag="ps")
                tr(ps, pc_rm[:, 2 * t:2 * t + 2])
                nc.scalar.activation(cA[0:2, 128 * t:128 * (t + 1)], ps, AF.Copy)
            for t in range(T):
                ps = pa_ps.tile([2, 128], F32, tag="ps")
                tr(ps, pco_rm[:, 2 * t:2 * t + 2])
                nc.scalar.activation(pcT_own[:, 128 * t:128 * (t + 1)], ps, AF.Copy)
            nc.vector.tensor_copy(rhs3[0:2, :], cA[0:2, :])
            ones21 = pa.tile([2, 1], F32, tag="ones21")
            nc.vector.memset(ones21, 1.0)
            # sq and r rows, chunked; -sq goes into rhs3[2:3] via metric sign trick
            for c in range(0, N, 512):
                ce = min(c + 512, N)
                sqch = pch.tile([2, 512], F32, tag="sqch")
                nc.vector.tensor_mul(sqch[:, 0:ce - c], cA[0:2, c:ce],
                                     cA[0:2, c:ce])
                ps = pa_ps.tile([1, 512], F32, tag="ps")
                nc.tensor.matmul(ps[:, 0:ce - c], ones21, sqch[:, 0:ce - c],
                                 start=True, stop=True)
                sqv = pch.tile([1, 512], F32, tag="sqv")
                nc.scalar.activation(sqv[:, 0:ce - c], ps[:, 0:ce - c], AF.Copy)
                nc.sync.dma_start(out=rhs3[2:3, c:ce], in_=sqv[:, 0:ce - c])
                rv = pch.tile([1, 512], F32, tag="rv")
                nc.scalar.activation(rv[:, 0:ce - c], ps[:, 0:ce - c], AF.Sqrt)
                nc.sync.dma_start(out=cA[2:3, c:ce], in_=rv[:, 0:ce - c])
            ones_dma(cA[3:4, :], N)

            cAo = pa.tile([4, R], F32, tag="cAo")
            nc.vector.tensor_copy(cAo[0:2, :], pcT_own)
            for c in range(0, R, 512):
                ce = min(c + 512, R)
                sqch = pch.tile([2, 512], F32, tag="sqch")
                nc.vector.tensor_mul(sqch[:, 0:ce - c], pcT_own[:, c:ce],
                                     pcT_own[:, c:ce])
                ps = pa_ps.tile([1, 512], F32, tag="ps")
                nc.tensor.matmul(ps[:, 0:ce - c], ones21, sqch[:, 0:ce - c],
                                 start=True, stop=True)
                rv = pch.tile([1, 512], F32, tag="rv")
                nc.scalar.activation(rv[:, 0:ce - c], ps[:, 0:ce - c], AF.Sqrt)
                nc.sync.dma_start(out=cAo[2:3, c:ce], in_=rv[:, 0:ce - c])
            ones_dma(cAo[3:4, :], R)

            # A0T full batch / A1T own
            a0t = pa.tile([32, N], F32, tag="a0t")
            a1to = pa.tile([64, R], F32, tag="a1to")
            for c in range(0, N, 512):
                ce = min(c + 512, N)
                ps = pa_ps.tile([32, 512], F32, tag="ps")
                nc.tensor.matmul(ps[:, 0:ce - c], wsb["w_a0"], cA[:, c:ce],
                                 start=True, stop=True)
                nc.scalar.activation(a0t[:, c:ce], ps[:, 0:ce - c], AF.Copy)
            for c in range(0, R, 512):
                ce = min(c + 512, R)
                ps = pa_ps.tile([64, 512], F32, tag="ps")
                nc.tensor.matmul(ps[:, 0:ce - c], wsb["w_a1"], cAo[:, c:ce],
                                 start=True, stop=True)
                nc.scalar.activation(a1to[:, c:ce], ps[:, 0:ce - c], AF.Copy)

            # table0 geometry columns: [x,y,x2,y2,xy,0,0,0 | A0(32)]
            tb0s = pa.tile([128, PT * 8], F32, tag="tb0s")
            nc.vector.memset(tb0s, 0.0)
            tsv = tb0s.rearrange("p (t c) -> p t c", c=8)
            prv = pc_rm.rearrange("p (t c) -> p t c", c=2)
            xv = prv[:, :, 0:1]
            yv = prv[:, :, 1:2]
            nc.vector.tensor_copy(tsv[:, :, 0:2], prv)
            nc.vector.tensor_mul(tsv[:, :, 2:3], xv, xv)
            nc.vector.tensor_mul(tsv[:, :, 3:4], yv, yv)
            nc.vector.tensor_mul(tsv[:, :, 4:5], xv, yv)
            nc.sync.dma_start(
                out=table0.ap().rearrange("(t p) c -> p t c", p=128)[:, :, 0:8],
                in_=tsv)
            zpad = pa.tile([128, 64], F32, tag="zpad")
            nc.vector.memset(zpad, 0.0)
            for t in range(PT):
                ps = pa_ps.tile([128, 32], F32, tag="psr")
                tr(ps, a0t[:, 128 * t:128 * (t + 1)])
                tbt = pch.tile([128, 32], F32, tag="tbt")
                nc.vector.tensor_copy(tbt, ps)
                nc.sync.dma_start(out=table0.ap()[128 * t:128 * (t + 1), 8:40],
                                  in_=tbt)
                nc.sync.dma_start(out=table0.ap()[128 * t:128 * (t + 1), 136:192],
                                  in_=zpad[:, 0:56])
            for t in range(T):
                ps = pa_ps.tile([128, 64], F32, tag="psr")
                tr(ps, a1to[:, 128 * t:128 * (t + 1)])
                tbt = pch.tile([128, 64], F32, tag="tbt1")
                nc.vector.tensor_copy(tbt, ps)
                nc.sync.dma_start(out=table1_own.ap()[128 * t:128 * (t + 1), 0:64],
                                  in_=tbt)

        # ---- A-feats scope ----
        with tc.tile_pool(name="paf", bufs=1) as pa, \
             tc.tile_pool(name="paf_ch", bufs=2) as pch, \
             tc.tile_pool(name="paf_ps", bufs=2, space="PSUM") as pa_ps:
            ftp = pa.tile([65, N], F32, tag="ftp")
            for t in range(PT):
                frt = pch.tile([128, 64], F32, tag="frt")
                nc.sync.dma_start(out=frt,
                                  in_=feats_b.ap()[128 * t:128 * (t + 1), :])
                ps = pa_ps.tile([64, 128], F32, tag="ps")
                tr(ps, frt)
                nc.scalar.activation(ftp[0:64, 128 * t:128 * (t + 1)], ps, AF.Copy)
            ones_dma(ftp[64:65, :], N)
            ftpo = pa.tile([65, R], F32, tag="ftpo")
            for t in range(T):
                frt = pch.tile([128, 64], F32, tag="frt")
                nc.sync.dma_start(out=frt,
                                  in_=feats_own.ap()[128 * t:128 * (t + 1), :])
                ps = pa_ps.tile([64, 128], F32, tag="ps")
                tr(ps, frt)
                nc.scalar.activation(ftpo[0:64, 128 * t:128 * (t + 1)], ps, AF.Copy)
            ones_dma(ftpo[64:65, :], R)

            for c in range(0, R, 512):
                ce = min(c + 512, R)
                ps = pa_ps.tile([128, 512], F32, tag="psm")
                nc.tensor.matmul(ps[:, 0:ce - c], wsb["w_res"], ftpo[:, c:ce],
                                 start=True, stop=True)
                nc.scalar.activation(y_resT[:, c:ce], ps[:, 0:ce - c], AF.Relu)

            x0tp = pa.tile([33, N], F32, tag="x0tp")
            for c in range(0, N, 512):
                ce = min(c + 512, N)
                ps = pa_ps.tile([32, 512], F32, tag="psm")
                nc.tensor.matmul(ps[:, 0:ce - c], wsb["w_mlp0"], ftp[:, c:ce],
                                 start=True, stop=True)
                nc.scalar.activation(x0tp[0:32, c:ce], ps[:, 0:ce - c], AF.Relu)
            ones_dma(x0tp[32:33, :], N)

            p0t = pa.tile([64, N], F32, tag="p0t")
            for c in range(0, N, 512):
                ce = min(c + 512, N)
                ps = pa_ps.tile([64, 512], F32, tag="psm")
                nc.tensor.matmul(ps[:, 0:ce - c], wsb["w_p0"], x0tp[:, c:ce],
                                 start=True, stop=True)
                nc.scalar.activation(p0t[:, c:ce], ps[:, 0:ce - c], AF.Copy)

            # table0 feature columns: x0 (40:72), P0 (72:136)
            for t in range(PT):
                ps = pa_ps.tile([128, 96], F32, tag="psr")
                sl = slice(128 * t, 128 * (t + 1))
                tr(ps[:, 0:32], x0tp[0:32, sl])
                tr(ps[:, 32:96], p0t[:, sl])
                tbt = pch.tile([128, 96], F32, tag="tbt")
                nc.vector.tensor_copy(tbt, ps)
                nc.sync.dma_start(out=table0.ap()[sl, 40:136], in_=tbt)

        # ================= PHASE B: KNN =================
        with tc.tile_pool(name="pb", bufs=2) as pb, \
             tc.tile_pool(name="pb_ps", bufs=2, space="PSUM") as pb_ps:
            for t in range(T):
                lhs3 = pb.tile([3, 128], F32, tag="lhs3")
                nc.vector.memset(lhs3, -1.0)
                nc.vector.tensor_scalar_mul(lhs3[0:2, :],
                                            pcT_own[:, 128 * t:128 * (t + 1)], 2.0)
                met = pb.tile([128, N], F32, tag="met")
                CW = min(N, 2048)
                for c in range(0, N, CW):
                    ps = pb_ps.tile([128, CW], F32, tag="ps_met")
                    for c2 in range(0, CW, 512):
                        nc.tensor.matmul(ps[:, c2:c2 + 512], lhs3,
                                         rhs3[:, c + c2:c + c2 + 512],
                                         start=True, stop=True)
                    nc.scalar.activation(met[:, c:c + CW], ps, AF.Copy)
                vals = pb.tile([128, 24], F32, tag="vals")
                ib = 24 * t
                if skip_topk:
                    nc.vector.memset(idxs_u16[:, ib:ib + 24], 0)
                    continue
                nc.vector.max(out=vals[:, 0:8], in_=met)
                nc.vector.max_index(idxs_u16[:, ib:ib + 8], vals[:, 0:8], met)
                nc.vector.match_replace(out=met, in_to_replace=vals[:, 0:8],
                                        in_values=met, imm_value=NEG)
                nc.vector.max(out=vals[:, 8:16], in_=met)
                nc.vector.max_index(idxs_u16[:, ib + 8:ib + 16], vals[:, 8:16], met)
                nc.vector.match_replace(out=met, in_to_replace=vals[:, 8:16],
                                        in_values=met, imm_value=NEG)
                nc.vector.max(out=vals[:, 16:24], in_=met)
                nc.vector.max_index(idxs_u16[:, ib + 16:ib + 24], vals[:, 16:24], met)

            # wrapped idx tensor for dma_gather
            nc.vector.memset(widx, 0)
            iv = idxs_u16.bitcast(I16).rearrange("p (t c) -> p t c", c=24)
            wv = widx.rearrange("p (t k r) -> p t k r", k=K1, r=8)
            for rh in range(8):
                for t in range(T):
                    nc.sync.dma_start(out=wv[0:16, t, :, rh],
                                      in_=iv[16 * rh:16 * rh + 16, t, 0:K1])
            for g in range(1, 8):
                nc.sync.dma_start(out=widx[16 * g:16 * (g + 1), :],
                                  in_=widx[0:16, :])

        # ================= PHASE C: stage 0 =================
        with tc.tile_pool(name="pc0", bufs=2) as pc0, \
             tc.tile_pool(name="pc0_ps", bufs=2, space="PSUM") as pc0_ps:
            for t in range(T):
                rec0 = pc0.tile([128, K1, REC0], F32, tag="rec0")
                if gather_mode == "swdge":
                    nc.gpsimd.dma_gather(
                        out_ap=rec0, in_ap=table0.ap(),
                        idxs_ap=widx[:, S17 * t:S17 * (t + 1)],
                        num_idxs=K1 * 128, num_idxs_reg=K1 * 128,
                        elem_size=REC0, single_packet=False)
                elif gather_mode == "fake":
                    nc.sync.dma_start(
                        out=rec0,
                        in_=table0.ap()[0:K1 * 128, :]
                            .rearrange("(k p) c -> p k c", p=128))
                else:
                    nc.vector.memset(rec0, 0.0)

                # coordsT [85, 128]
                co5 = pc0.tile([128, K1 * 5], F32, tag="co5")
                nc.vector.tensor_copy(co5.rearrange("p (k c) -> p k c", c=5),
                                      rec0[:, :, 0:5])
                psc = pc0_ps.tile([85, 128], F32, tag="pssm")
                tr(psc, co5)
                coT = pc0.tile([85, 128], F32, tag="coT")
                nc.scalar.activation(coT, psc, AF.Copy)

                # S + stats [128, 101]
                pss = pc0_ps.tile([128, 101], F32, tag="pssm")
                nc.tensor.matmul(pss, coT, wsb["w_ss"], start=True, stop=True)
                sall = pc0.tile([128, 101], F32, tag="sall")
                nc.vector.tensor_copy(sall, pss)
                nc.vector.tensor_copy(S1_keep[:, 64 * t:64 * (t + 1)],
                                      sall[:, 37:101])

                # ggf
                mn = pc0.tile([128, 5], F32, tag="mn")
                nc.vector.tensor_scalar_mul(mn, sall[:, 32:37], 1.0 / K1)
                gg = pc0.tile([128, 8], F32, tag="gg")
                # gg cols: 0 cov, 1 vx, 2 vy, 3 tmp, 4 m, 5 pdist
                nc.vector.tensor_mul(gg[:, 3:4], mn[:, 0:1], mn[:, 1:2])
                nc.vector.tensor_sub(gg[:, 0:1], mn[:, 4:5], gg[:, 3:4])
                nc.vector.tensor_mul(gg[:, 3:4], mn[:, 0:1], mn[:, 0:1])
                nc.vector.tensor_sub(gg[:, 1:2], mn[:, 2:3], gg[:, 3:4])
                nc.vector.tensor_mul(gg[:, 3:4], mn[:, 1:2], mn[:, 1:2])
                nc.vector.tensor_sub(gg[:, 2:3], mn[:, 3:4], gg[:, 3:4])
                nc.vector.tensor_scalar_add(gg[:, 3:4], gg[:, 1:2], EPS)
                nc.vector.reciprocal(gg[:, 3:4], gg[:, 3:4])
                nc.vector.tensor_mul(gg[:, 4:5], gg[:, 0:1], gg[:, 3:4])
                nc.vector.tensor_mul(gg[:, 3:4], gg[:, 1:2], gg[:, 2:3])
                nc.vector.tensor_scalar_add(gg[:, 3:4], gg[:, 3:4], EPS)
                nc.scalar.activation(gg[:, 3:4], gg[:, 3:4], AF.Sqrt)
                nc.vector.reciprocal(gg[:, 3:4], gg[:, 3:4])
                nc.vector.tensor_mul(gg[:, 5:6], gg[:, 0:1], gg[:, 3:4])
                nc.vector.tensor_scalar(gg[:, 5:6], gg[:, 5:6], -1.0, 1.0,
                                        op0=mybir.AluOpType.mult,
                                        op1=mybir.AluOpType.add)
                nc.vector.tensor_copy(ggf_keep[:, 2 * t:2 * t + 2], gg[:, 4:6])

                # r0 = relu(A0slab + S0)
                r0 = pc0.tile([128, K1 * 32], F32, tag="r0")
                r0v = r0.rearrange("p (k c) -> p k c", c=32)
                nc.vector.tensor_add(r0v, rec0[:, :, 8:40],
                                     sall[:, 0:32].rearrange("p (u c) -> p u c", u=1).to_broadcast([128, K1, 32]))
                nc.vector.tensor_scalar_max(r0, r0, 0.0)

                # r0T [32, K1*128] via chunked 1-bank psum tiles
                r0T = pc0.tile([32, K1 * 128], F32, tag="r0T")
                for k in range(K1):
                    psk = pc0_ps.tile([128, 512], F32, tag="pst", bufs=6)
                    tr(psk[0:32, 0:128], r0[:, 32 * k:32 * (k + 1)])
                    nc.scalar.activation(r0T[:, 128 * k:128 * (k + 1)],
                                         psk[0:32, 0:128], AF.Copy)

                # s_rT [64, K1*128]
                srT = pc0.tile([64, K1 * 128], F32, tag="srT")
                for c in range(0, K1 * 128, 512):
                    ce = min(c + 512, K1 * 128)
                    psk = pc0_ps.tile([128, 512], F32, tag="pst", bufs=6)
                    nc.tensor.matmul(psk[0:64, 0:ce - c], wsb["w_sr0"],
                                     r0T[:, c:ce], start=True, stop=True)
                    nc.scalar.activation(srT[:, c:ce], psk[0:64, 0:ce - c],
                                         AF.Copy)

                # back-transpose chunks into SBUF slab + add P0 slab
                sl = pc0.tile([128, K1 * 64], F32, tag="slsb")
                for k in range(K1):
                    psk = pc0_ps.tile([128, 512], F32, tag="pst", bufs=6)
                    tr(psk[:, 0:64], srT[:, 128 * k:128 * (k + 1)])
                    nc.scalar.activation(sl[:, 64 * k:64 * (k + 1)],
                                         psk[:, 0:64], AF.Copy)
                ssb = pc0.tile([128, K1 * 64], F32, tag="ssb")
                nc.vector.tensor_add(ssb.rearrange("p (k c) -> p k c", c=64),
                                     sl.rearrange("p (k c) -> p k c", c=64),
                                     rec0[:, :, 72:136])
                esb = pc0.tile([128, K1 * 64], F32, tag="esb")
                nc.scalar.activation(esb, ssb, AF.Exp)
                zz = pc0.tile([128, K1], F32, tag="zz")
                nc.vector.tensor_reduce(zz, esb.rearrange("p (k c) -> p k c", c=64),
                                        axis=mybir.AxisListType.X,
                                        op=mybir.AluOpType.add)
                rz = pc0.tile([128, K1], F32, tag="rz")
                nc.vector.reciprocal(rz, zz)
                sm = pc0.tile([128, K1, 64], F32, tag="sm")
                nc.vector.tensor_mul(sm, esb.rearrange("p (k c) -> p k c", c=64),
                                     rz.rearrange("p (k u) -> p k u", u=1)
                                       .to_broadcast([128, K1, 64]))
                # attention
                pr = pc0.tile([128, K1, 32], F32, tag="pr")
                att = pc0.tile([128, 67], F32, tag="att")
                nc.gpsimd.tensor_mul(pr, sm[:, :, 0:32], rec0[:, :, 40:72])
                nc.vector.tensor_reduce(
                    att[:, 0:32], pr.rearrange("p k c -> p c k"),
                    axis=mybir.AxisListType.X, op=mybir.AluOpType.add)
                nc.vector.tensor_mul(pr, sm[:, :, 32:64],
                                     r0.rearrange("p (k c) -> p k c", c=32))
                nc.vector.tensor_reduce(
                    att[:, 32:64], pr.rearrange("p k c -> p c k"),
                    axis=mybir.AxisListType.X, op=mybir.AluOpType.add)
                nc.vector.tensor_copy(att[:, 64:66], gg[:, 4:6])
                nc.vector.memset(att[:, 66:67], 1.0)

                # x1 chunk = relu(w_f0^T att)
                psa = pc0_ps.tile([67, 128], F32, tag="pssm")
                tr(psa, att)
                attT = pc0.tile([67, 128], F32, tag="attT")
                nc.scalar.activation(attT, psa, AF.Copy)
                psx = pc0_ps.tile([64, 128], F32, tag="pssm")
                nc.tensor.matmul(psx, wsb["w_f0"], attT, start=True, stop=True)
                nc.scalar.activation(x1T[0:64, 128 * t:128 * (t + 1)], psx, AF.Relu)

            for c in range(0, R, min(N, 2048)):
                ce = min(c + min(N, 2048), R)
                nc.sync.dma_start(out=x1T[64:65, c:ce],
                                  in_=wt["ones_n"].ap()[:, 0:ce - c])
            # P1T + table1 records
            p1t = pc0.tile([128, R], F32, tag="p1t")
            for c in range(0, R, 512):
                ce = min(c + 512, R)
                ps = pc0_ps.tile([128, 512], F32, tag="pssm")
                nc.tensor.matmul(ps[:, 0:ce - c], wsb["w_p1"], x1T[:, c:ce],
                                 start=True, stop=True)
                nc.scalar.activation(p1t[:, c:ce], ps[:, 0:ce - c], AF.Copy)
            for t in range(T):
                ps = pc0_ps.tile([128, 192], F32, tag="pssm")
                sl = slice(128 * t, 128 * (t + 1))
                tr(ps[:, 0:64], x1T[0:64, sl])
                tr(ps[:, 64:192], p1t[:, sl])
                tbt1 = pc0.tile([128, 192], F32, tag="tbt1")
                nc.vector.tensor_copy(tbt1, ps)
                nc.sync.dma_start(out=table1_own.ap()[sl, 64:256], in_=tbt1)

        # ---- AllGather ----
        if no_cc:
            nc.sync.dma_start(out=table1.ap()[0:R, :], in_=table1_own.ap())
        elif CPB > 1:
            nc.gpsimd.collective_compute(
                "AllGather", mybir.AluOpType.bypass,
                replica_groups=groups,
                ins=[table1_own.ap()], outs=[table1.ap()])
        else:
            nc.sync.dma_start(out=table1.ap(), in_=table1_own.ap())

        # ================= PHASE D: stage 1 + final =================
        with tc.tile_pool(name="pd", bufs=2) as pd, \
             tc.tile_pool(name="pd_ps", bufs=2, space="PSUM") as pd_ps:
            for t in range(T):
                rec1 = pd.tile([128, K1, REC1], F32, tag="rec1")
                if gather_mode == "swdge":
                    nc.gpsimd.dma_gather(
                        out_ap=rec1, in_ap=table1.ap(),
                        idxs_ap=widx[:, S17 * t:S17 * (t + 1)],
                        num_idxs=K1 * 128, num_idxs_reg=K1 * 128,
                        elem_size=REC1, single_packet=False)
                elif gather_mode == "fake":
                    nc.sync.dma_start(
                        out=rec1,
                        in_=table1.ap()[0:K1 * 128, :]
                            .rearrange("(k p) c -> p k c", p=128))
                else:
                    nc.vector.memset(rec1, 0.0)

                r1 = pd.tile([128, K1 * 64], F32, tag="r1")
                nc.vector.tensor_add(
                    r1.rearrange("p (k c) -> p k c", c=64), rec1[:, :, 0:64],
                    S1_keep[:, 64 * t:64 * (t + 1)].rearrange("p (u c) -> p u c", u=1).to_broadcast([128, K1, 64]))
                nc.vector.tensor_scalar_max(r1, r1, 0.0)

                r1T = pd.tile([64, K1 * 128], F32, tag="r1T")
                for k in range(K1):
                    psk = pd_ps.tile([128, 512], F32, tag="pst1", bufs=6)
                    tr(psk[0:64, 0:128], r1[:, 64 * k:64 * (k + 1)])
                    nc.scalar.activation(r1T[:, 128 * k:128 * (k + 1)],
                                         psk[0:64, 0:128], AF.Copy)

                srT = pd.tile([128, K1 * 128], F32, tag="sr1T")
                for c in range(0, K1 * 128, 512):
                    ce = min(c + 512, K1 * 128)
                    psk = pd_ps.tile([128, 512], F32, tag="pst1", bufs=6)
                    nc.tensor.matmul(psk[:, 0:ce - c], wsb["w_sr1"],
                                     r1T[:, c:ce], start=True, stop=True)
                    nc.scalar.activation(srT[:, c:ce], psk[:, 0:ce - c], AF.Copy)

                sl = pd.tile([128, K1 * 128], F32, tag="slsb1")
                for k in range(K1):
                    psk = pd_ps.tile([128, 512], F32, tag="pst1", bufs=6)
                    tr(psk[:, 0:128], srT[:, 128 * k:128 * (k + 1)])
                    nc.scalar.activation(sl[:, 128 * k:128 * (k + 1)],
                                         psk[:, 0:128], AF.Copy)
                nc.vector.tensor_add(sl.rearrange("p (k c) -> p k c", c=128),
                                     sl.rearrange("p (k c) -> p k c", c=128),
                                     rec1[:, :, 128:256])
                esb = pd.tile([128, K1 * 128], F32, tag="esb1", bufs=1)
                nc.scalar.activation(esb, sl, AF.Exp)
                zz = pd.tile([128, K1], F32, tag="zz1")
                nc.vector.tensor_reduce(zz, esb.rearrange("p (k c) -> p k c", c=128),
                                        axis=mybir.AxisListType.X,
                                        op=mybir.AluOpType.add)
                rz = pd.tile([128, K1], F32, tag="rz1")
                nc.vector.reciprocal(rz, zz)
                sm = pd.tile([128, K1, 128], F32, tag="sm1", bufs=1)
                nc.vector.tensor_mul(sm, esb.rearrange("p (k c) -> p k c", c=128),
                                     rz.rearrange("p (k u) -> p k u", u=1)
                                       .to_broadcast([128, K1, 128]))
                pr = pd.tile([128, K1, 64], F32, tag="pr1")
                att = pd.tile([128, 128], F32, tag="att1")
                nc.gpsimd.tensor_mul(pr, sm[:, :, 0:64], rec1[:, :, 64:128])
                nc.vector.tensor_reduce(
                    att[:, 0:64], pr.rearrange("p k c -> p c k"),
                    axis=mybir.AxisListType.X, op=mybir.AluOpType.add)
                nc.vector.tensor_mul(pr, sm[:, :, 64:128],
                                     r1.rearrange("p (k c) -> p k c", c=64))
                nc.vector.tensor_reduce(
                    att[:, 64:128], pr.rearrange("p k c -> p c k"),
                    axis=mybir.AxisListType.X, op=mybir.AluOpType.add)

                # x2 = relu(Wf1a^T att1 + Wf1b^T [ggf; 1] + bf1)
                psa = pd_ps.tile([128, 128], F32, tag="pssm1")
                tr(psa, att)
                attT = pd.tile([128, 128], F32, tag="attT1")
                nc.scalar.activation(attT, psa, AF.Copy)
                gco = pd.tile([128, 3], F32, tag="gco")
                nc.vector.tensor_copy(gco[:, 0:2], ggf_keep[:, 2 * t:2 * t + 2])
                nc.vector.memset(gco[:, 2:3], 1.0)
                psg = pd_ps.tile([3, 128], F32, tag="pssm1")
                tr(psg, gco)
                gcoT = pd.tile([3, 128], F32, tag="gcoT")
                nc.scalar.activation(gcoT, psg, AF.Copy)
                psx = pd_ps.tile([128, 128], F32, tag="pssm1")
                nc.tensor.matmul(psx, wsb["w_f1a"], attT, start=True, stop=False)
                nc.tensor.matmul(psx, wsb["w_f1b"], gcoT, start=False, stop=True)
                x2T = pd.tile([128, 128], F32, tag="x2T")
                nc.scalar.activation(x2T, psx, AF.Relu)

                # x3 = relu(W1^T x2 + b1); out = leaky(x3 + y, 0.2)
                pso = pd_ps.tile([128, 128], F32, tag="pssm1")
                nc.tensor.matmul(pso, wsb["w_out"], x2T, start=True, stop=True)
                x3T = pd.tile([128, 128], F32, tag="x3T")
                nc.scalar.activation(x3T, pso, AF.Relu, bias=wsb["b_out"][:, 0:1])
                nc.vector.tensor_add(x3T, x3T, y_resT[:, 128 * t:128 * (t + 1)])
                pos = pd.tile([128, 128], F32, tag="pos")
                nc.vector.tensor_scalar_max(pos, x3T, 0.0)
                nc.vector.tensor_scalar(x3T, x3T, 0.0, 0.2,
                                        op0=mybir.AluOpType.min,
                                        op1=mybir.AluOpType.mult)
                nc.vector.tensor_add(x3T, x3T, pos)

                psf = pd_ps.tile([128, 128], F32, tag="pssm1")
                tr(psf, x3T)
                orow = pd.tile([128, 128], F32, tag="orow")
                nc.scalar.activation(orow, psf, AF.Copy)
                nc.sync.dma_start(out=out.ap()[128 * t:128 * (t + 1), :], in_=orow)

        pp.release()

    nc.compile()
    return nc


_CACHE = {}


def _get_module(N, CPB, n_cores):
    key = (N, CPB, n_cores)
    if key not in _CACHE:
        _CACHE[key] = build_module(N, CPB, n_cores)
    return _CACHE[key]


def kernel(**inputs):
    pc = np.asarray(inputs["pc"])
    feats = np.asarray(inputs["feats"])
    Bv, N, _ = pc.shape
    n_cores = 8
    CPB = n_cores // Bv
    R = N // CPB
    nc = _get_module(N, CPB, n_cores)
    wts = host_weights(inputs, N)
    in_maps = []
    for c in range(n_cores):
        b = c // CPB
        s = (c % CPB) * R
        m = {"pc_b": np.ascontiguousarray(pc[b]),
             "feats_b": np.ascontiguousarray(feats[b]),
             "pc_own": np.ascontiguousarray(pc[b, s:s + R]),
             "feats_own": np.ascontiguousarray(feats[b, s:s + R])}
        m.update(wts)
        in_maps.append(m)
    res = run_bass_kernel_spmd(nc, in_maps, list(range(n_cores)))
    out = np.empty((Bv, N, 128), np.float32)
    for c in range(n_cores):
        b = c // CPB
        s = (c % CPB) * R
        out[b, s:s + R] = res.results[c]["out"]
    return out



# revision 9
# speedup vs baseline: 2.2299x; 2.2299x over previous
"""Bass/Trainium2 kernel for nn_DilatedResBlock (RandLA-Net style block).

Sharding: batch B=2 x 4-way split of N=8192 points -> 8 cores.
Each core: KNN top-17 for its 2048 rows (full-batch candidate scan via PE
matmul metric + VectorE max8/match_replace rounds), geometry encoding via
algebraic fold (per-point A-tables + per-row S-matmul), two attentive
pooling stages with per-neighbor record gathers (SWDGE dma_gather), one
AllGather of stage-0 outputs across the 4 cores of a batch.
"""

import numpy as np

import concourse.bass as bass
import concourse.mybir as mybir
from concourse import bacc
from concourse.bass_utils import run_bass_kernel_spmd
from concourse.tile import TileContext

F32 = mybir.dt.float32
U16 = mybir.dt.uint16
I16 = mybir.dt.int16

B, DIMS, K1 = 2, 2, 17
IN_UNITS, UNITS = 64, 128
EPS = 1e-8
NEG = -3.0e38

AF = mybir.ActivationFunctionType


def host_weights(w, N):
    """Host-side rearrangements of the given weights (no data-dependent work)."""
    f = np.float32
    Wl0, bl0 = w["Wl0"], w["bl0"]
    Wl1, bl1 = w["Wl1"], w["bl1"]
    # A-fold: r0_pre[n,j] = [x_j,y_j,r_j,1] @ w_a + S[n]
    def afold(Wl, bl):
        xco = Wl[0] - Wl[2::3].sum(0)
        yco = Wl[1] - Wl[3::3].sum(0)
        rco = Wl[4::3].sum(0)
        return np.stack([xco, yco, rco, bl]).astype(f)

    w_a0 = afold(Wl0, bl0)          # [4, 32]
    w_a1 = afold(Wl1, bl1)          # [4, 64]

    # S + stats matmul weights: coords5 channel (k, j), j in {x,y,x2,y2,xy}
    w_ss = np.zeros((5 * K1, 32 + 5 + 64), f)
    for k in range(K1):
        w_ss[5 * k + 0, 0:32] = Wl0[2 + 3 * k]
        w_ss[5 * k + 1, 0:32] = Wl0[3 + 3 * k]
        w_ss[5 * k + 0, 32] = 1.0
        w_ss[5 * k + 1, 33] = 1.0
        w_ss[5 * k + 2, 34] = 1.0
        w_ss[5 * k + 3, 35] = 1.0
        w_ss[5 * k + 4, 36] = 1.0
        w_ss[5 * k + 0, 37:101] = Wl1[2 + 3 * k]
        w_ss[5 * k + 1, 37:101] = Wl1[3 + 3 * k]

    vs = lambda W, b: np.vstack([W, b[None, :]]).astype(f)
    return {
        "w_a0": w_a0, "w_a1": w_a1, "w_ss": w_ss.astype(f),
        "w_mlp0": vs(w["W0"], w["b0"]),          # [65, 32]
        "w_res": vs(w["W_res"], w["b_res"]),     # [65, 128]
        "w_p0": vs(w["Ws0"][0:32], w["bs0"]),    # [33, 64]
        "w_sr0": w["Ws0"][32:64].astype(f),      # [32, 64]
        "w_p1": vs(w["Ws1"][0:64], w["bs1"]),    # [65, 128]
        "w_sr1": w["Ws1"][64:128].astype(f),     # [64, 128]
        "w_f0": vs(w["Wf0"], w["bf0"]),          # [67, 64]
        "w_f1a": w["Wf1"][0:128].astype(f),      # [128, 128]
        "w_f1b": vs(w["Wf1"][128:130], w["bf1"]),# [3, 128]
        "w_out": w["W1"].astype(f),              # [128, 128]
        "b_out": w["b1"].reshape(128, 1).astype(f),
        "ident": np.eye(128, dtype=f),
        "ones_n": np.ones((1, min(N, 2048)), f),
    }


WNAMES = ["w_a0", "w_a1", "w_ss", "w_mlp0", "w_res", "w_p0", "w_sr0",
          "w_p1", "w_sr1", "w_f0", "w_f1a", "w_f1b", "w_out", "b_out",
          "ident", "ones_n"]
WSHAPES = None  # filled in build


def build_module(N, CPB, n_cores, no_cc=False, skip_topk=False,
                 gather_mode="swdge", skip_stage0=False, skip_stage1=False):
    """Build the SPMD Bass module. R = N // CPB own rows per core."""
    R = N // CPB
    T = R // 128      # own row tiles
    PT = N // 128     # full-batch point tiles
    REC0 = 192
    REC1 = 256
    S17 = 136         # wrapped idx cols per tile (17*8)

    nc = bacc.Bacc("TRN2", target_bir_lowering=False, debug=False,
                   num_devices=n_cores)

    # --- I/O ---
    pc_b = nc.dram_tensor("pc_b", [N, 2], F32, kind="ExternalInput")
    feats_b = nc.dram_tensor("feats_b", [N, 64], F32, kind="ExternalInput")
    pc_own = nc.dram_tensor("pc_own", [R, 2], F32, kind="ExternalInput")
    feats_own = nc.dram_tensor("feats_own", [R, 64], F32, kind="ExternalInput")
    wt = {}
    shapes = {"w_a0": [4, 32], "w_a1": [4, 64], "w_ss": [85, 101],
              "w_mlp0": [65, 32], "w_res": [65, 128], "w_p0": [33, 64],
              "w_sr0": [32, 64], "w_p1": [65, 128], "w_sr1": [64, 128],
              "w_f0": [67, 64], "w_f1a": [128, 128], "w_f1b": [3, 128],
              "w_out": [128, 128], "b_out": [128, 1],
              "ident": [128, 128], "ones_n": [1, min(N, 2048)]}
    for nm in WNAMES:
        wt[nm] = nc.dram_tensor(nm, shapes[nm], F32, kind="ExternalInput")
    out = nc.dram_tensor("out", [R, 128], F32, kind="ExternalOutput")

    table0 = nc.dram_tensor("table0", [N, REC0], F32)
    table1_own = nc.dram_tensor("table1_own", [R, REC1], F32)
    table1 = nc.dram_tensor("table1", [N, REC1], F32)

    groups = [[g * CPB + j for j in range(CPB)] for g in range(n_cores // CPB)]

    with TileContext(nc) as tc:
        # ---- persistent pool (lives whole kernel) ----
        pp = tc.alloc_tile_pool(name="persist", bufs=1)
        wsb = {nm: pp.tile(shapes[nm], F32, tag=nm, name=nm) for nm in WNAMES}
        for nm in WNAMES:
            nc.sync.dma_start(out=wsb[nm], in_=wt[nm].ap())
        ident = wsb["ident"]

        def tr(out_ap, in_ap):
            k = in_ap.shape[0]
            nc.tensor.transpose(out_ap, in_ap, ident[0:k, 0:k])

        rhs3 = pp.tile([3, N], F32, tag="rhs3")
        pcT_own = pp.tile([2, R], F32, tag="pcT_own")
        y_resT = pp.tile([128, R], F32, tag="y_resT")
        S1_keep = pp.tile([128, T * 64], F32, tag="S1_keep")
        ggf_keep = pp.tile([128, T * 2], F32, tag="ggf_keep")
        x1T = pp.tile([65, R], F32, tag="x1T")
        idxs_u16 = pp.tile([128, T * 24], U16, tag="idxs_u16")
        widx = pp.tile([128, T * S17], I16, tag="widx")

        # ================= PHASE A: tables =================
        OC = min(N, 2048)

        def ones_dma(dst_ap, length):
            for c in range(0, length, OC):
                ce = min(c + OC, length)
                nc.sync.dma_start(out=dst_ap[:, c:ce],
                                  in_=wt["ones_n"].ap()[:, 0:ce - c])

        # ---- A-geom scope ----
        with tc.tile_pool(name="pag", bufs=1) as pa, \
             tc.tile_pool(name="pag_ch", bufs=2) as pch, \
             tc.tile_pool(name="pa_ps", bufs=3, space="PSUM") as pa_ps:
            pc_rm = pa.tile([128, PT * 2], F32, tag="pc_rm")
            nc.sync.dma_start(out=pc_rm.rearrange("p (t c) -> p t c", c=2),
                              in_=pc_b.ap().rearrange("(t p) c -> p t c", p=128))
            pco_rm = pa.tile([128, T * 2], F32, tag="pco_rm")
            nc.sync.dma_start(out=pco_rm.rearrange("p (t c) -> p t c", c=2),
                              in_=pc_own.ap().rearrange("(t p) c -> p t c", p=128))

            # cA = [x; y; r; 1] channel-major, full batch
            cA = pa.tile([4, N], F32, tag="cA")
            for t in range(PT):
                ps = pa_ps.tile([2, 128], F32, tag="ps")
                tr(ps, pc_rm[:, 2 * t:2 * t + 2])
                nc.scalar.activation(cA[0:2, 128 * t:128 * (t + 1)], ps, AF.Copy)
            for t in range(T):
                ps = pa_ps.tile([2, 128], F32, tag="ps")
                tr(ps, pco_rm[:, 2 * t:2 * t + 2])
                nc.scalar.activation(pcT_own[:, 128 * t:128 * (t + 1)], ps, AF.Copy)
            nc.vector.tensor_copy(rhs3[0:2, :], cA[0:2, :])
            ones21 = pa.tile([2, 1], F32, tag="ones21")
            nc.vector.memset(ones21, 1.0)
            # sq and r rows, chunked; -sq goes into rhs3[2:3] via metric sign trick
            for c in range(0, N, 512):
                ce = min(c + 512, N)
                sqch = pch.tile([2, 512], F32, tag="sqch")
                nc.vector.tensor_mul(sqch[:, 0:ce - c], cA[0:2, c:ce],
                                     cA[0:2, c:ce])
                ps = pa_ps.tile([1, 512], F32, tag="ps")
                nc.tensor.matmul(ps[:, 0:ce - c], ones21, sqch[:, 0:ce - c],
                                 start=True, stop=True)
                sqv = pch.tile([1, 512], F32, tag="sqv")
                nc.scalar.activation(sqv[:, 0:ce - c], ps[:, 0:ce - c], AF.Copy)
                nc.sync.dma_start(out=rhs3[2:3, c:ce], in_=sqv[:, 0:ce - c])
                rv = pch.tile([1, 512], F32, tag="rv")
                nc.scalar.activation(rv[:, 0:ce - c], ps[:, 0:ce - c], AF.Sqrt)
                nc.sync.dma_start(out=cA[2:3, c:ce], in_=rv[:, 0:ce - c])
            ones_dma(cA[3:4, :], N)

            cAo = pa.tile([4, R], F32, tag="cAo")
            nc.vector.tensor_copy(cAo[0:2, :], pcT_own)
            for c in range(0, R, 512):
                ce = min(c + 512, R)
                sqch = pch.tile([2, 512], F32, tag="sqch")
                nc.vector.tensor_mul(sqch[:, 0:ce - c], pcT_own[:, c:ce],
                                     pcT_own[:, c:ce])
                ps = pa_ps.tile([1, 512], F32, tag="ps")
                nc.tensor.matmul(ps[:, 0:ce - c], ones21, sqch[:, 0:ce - c],
                                 start=True, stop=True)
                rv = pch.tile([1, 512], F32, tag="rv")
                nc.scalar.activation(rv[:, 0:ce - c], ps[:, 0:ce - c], AF.Sqrt)
                nc.sync.dma_start(out=cAo[2:3, c:ce], in_=rv[:, 0:ce - c])
            ones_dma(cAo[3:4, :], R)

            # A0T full batch / A1T own
            a0t = pa.tile([32, N], F32, tag="a0t")
            a1to = pa.tile([64, R], F32, tag="a1to")
            for c in range(0, N, 512):
                ce = min(c + 512, N)
                ps = pa_ps.tile([32, 512], F32, tag="ps")
                nc.tensor.matmul(ps[:, 0:ce - c], wsb["w_a0"], cA[:, c:ce],
                                 start=True, stop=True)
                nc.scalar.activation(a0t[:, c:ce], ps[:, 0:ce - c], AF.Copy)
            for c in range(0, R, 512):
                ce = min(c + 512, R)
                ps = pa_ps.tile([64, 512], F32, tag="ps")
                nc.tensor.matmul(ps[:, 0:ce - c], wsb["w_a1"], cAo[:, c:ce],
                                 start=True, stop=True)
                nc.scalar.activation(a1to[:, c:ce], ps[:, 0:ce - c], AF.Copy)

            # table0 geometry columns: [x,y,x2,y2,xy,0,0,0 | A0(32)]
            tb0s = pa.tile([128, PT * 8], F32, tag="tb0s")
            nc.vector.memset(tb0s, 0.0)
            tsv = tb0s.rearrange("p (t c) -> p t c", c=8)
            prv = pc_rm.rearrange("p (t c) -> p t c", c=2)
            xv = prv[:, :, 0:1]
            yv = prv[:, :, 1:2]
            nc.vector.tensor_copy(tsv[:, :, 0:2], prv)
            nc.vector.tensor_mul(tsv[:, :, 2:3], xv, xv)
            nc.vector.tensor_mul(tsv[:, :, 3:4], yv, yv)
            nc.vector.tensor_mul(tsv[:, :, 4:5], xv, yv)
            nc.sync.dma_start(
                out=table0.ap().rearrange("(t p) c -> p t c", p=128)[:, :, 0:8],
                in_=tsv)
            zpad = pa.tile([128, 64], F32, tag="zpad")
            nc.vector.memset(zpad, 0.0)
            for t in range(PT):
                ps = pa_ps.tile([128, 32], F32, tag="psr")
                tr(ps, a0t[:, 128 * t:128 * (t + 1)])
                tbt = pch.tile([128, 32], F32, tag="tbt")
                nc.vector.tensor_copy(tbt, ps)
                nc.sync.dma_start(out=table0.ap()[128 * t:128 * (t + 1), 8:40],
                                  in_=tbt)
                nc.sync.dma_start(out=table0.ap()[128 * t:128 * (t + 1), 136:192],
                                  in_=zpad[:, 0:56])
            for t in range(T):
                ps = pa_ps.tile([128, 64], F32, tag="psr")
                tr(ps, a1to[:, 128 * t:128 * (t + 1)])
                tbt = pch.tile([128, 64], F32, tag="tbt1")
                nc.vector.tensor_copy(tbt, ps)
                nc.sync.dma_start(out=table1_own.ap()[128 * t:128 * (t + 1), 0:64],
                                  in_=tbt)

        # ---- A-feats scope ----
        with tc.tile_pool(name="paf", bufs=1) as pa, \
             tc.tile_pool(name="paf_ch", bufs=2) as pch, \
             tc.tile_pool(name="paf_ps", bufs=2, space="PSUM") as pa_ps:
            ftp = pa.tile([65, N], F32, tag="ftp")
            for t in range(PT):
                frt = pch.tile([128, 64], F32, tag="frt")
                nc.sync.dma_start(out=frt,
                                  in_=feats_b.ap()[128 * t:128 * (t + 1), :])
                ps = pa_ps.tile([64, 128], F32, tag="ps")
                tr(ps, frt)
                nc.scalar.activation(ftp[0:64, 128 * t:128 * (t + 1)], ps, AF.Copy)
            ones_dma(ftp[64:65, :], N)
            ftpo = pa.tile([65, R], F32, tag="ftpo")
            for t in range(T):
                frt = pch.tile([128, 64], F32, tag="frt")
                nc.sync.dma_start(out=frt,
                                  in_=feats_own.ap()[128 * t:128 * (t + 1), :])
                ps = pa_ps.tile([64, 128], F32, tag="ps")
                tr(ps, frt)
                nc.scalar.activation(ftpo[0:64, 128 * t:128 * (t + 1)], ps, AF.Copy)
            ones_dma(ftpo[64:65, :], R)

            for c in range(0, R, 512):
                ce = min(c + 512, R)
                ps = pa_ps.tile([128, 512], F32, tag="psm")
                nc.tensor.matmul(ps[:, 0:ce - c], wsb["w_res"], ftpo[:, c:ce],
                                 start=True, stop=True)
                nc.scalar.activation(y_resT[:, c:ce], ps[:, 0:ce - c], AF.Relu)

            x0tp = pa.tile([33, N], F32, tag="x0tp")
            for c in range(0, N, 512):
                ce = min(c + 512, N)
                ps = pa_ps.tile([32, 512], F32, tag="psm")
                nc.tensor.matmul(ps[:, 0:ce - c], wsb["w_mlp0"], ftp[:, c:ce],
                                 start=True, stop=True)
                nc.scalar.activation(x0tp[0:32, c:ce], ps[:, 0:ce - c], AF.Relu)
            ones_dma(x0tp[32:33, :], N)

            p0t = pa.tile([64, N], F32, tag="p0t")
            for c in range(0, N, 512):
                ce = min(c + 512, N)
                ps = pa_ps.tile([64, 512], F32, tag="psm")
                nc.tensor.matmul(ps[:, 0:ce - c], wsb["w_p0"], x0tp[:, c:ce],
                                 start=True, stop=True)
                nc.scalar.activation(p0t[:, c:ce], ps[:, 0:ce - c], AF.Copy)

            # table0 feature columns: x0 (40:72), P0 (72:136)
            for t in range(PT):
                ps = pa_ps.tile([128, 96], F32, tag="psr")
                sl = slice(128 * t, 128 * (t + 1))
                tr(ps[:, 0:32], x0tp[0:32, sl])
                tr(ps[:, 32:96], p0t[:, sl])
                tbt = pch.tile([128, 96], F32, tag="tbt")
                nc.vector.tensor_copy(tbt, ps)
                nc.sync.dma_start(out=table0.ap()[sl, 40:136], in_=tbt)

        # ================= PHASE B: KNN =================
        with tc.tile_pool(name="pb", bufs=2) as pb, \
             tc.tile_pool(name="pb_ps", bufs=2, space="PSUM") as pb_ps:
            for t in range(T):
                lhs3 = pb.tile([3, 128], F32, tag="lhs3")
                nc.vector.memset(lhs3, -1.0)
                nc.vector.tensor_scalar_mul(lhs3[0:2, :],
                                            pcT_own[:, 128 * t:128 * (t + 1)], 2.0)
                met = pb.tile([128, N], F32, tag="met")
                CW = min(N, 2048)
                for c in range(0, N, CW):
                    ps = pb_ps.tile([128, CW], F32, tag="ps_met")
                    for c2 in range(0, CW, 512):
                        nc.tensor.matmul(ps[:, c2:c2 + 512], lhs3,
                                         rhs3[:, c + c2:c + c2 + 512],
                                         start=True, stop=True)
                    nc.scalar.activation(met[:, c:c + CW], ps, AF.Copy)
                vals = pb.tile([128, 24], F32, tag="vals")
                ib = 24 * t
                if skip_topk:
                    nc.vector.memset(idxs_u16[:, ib:ib + 24], 0)
                    continue
                nc.vector.max(out=vals[:, 0:8], in_=met)
                nc.vector.max_index(idxs_u16[:, ib:ib + 8], vals[:, 0:8], met)
                nc.vector.match_replace(out=met, in_to_replace=vals[:, 0:8],
                                        in_values=met, imm_value=NEG)
                nc.vector.max(out=vals[:, 8:16], in_=met)
                nc.vector.max_index(idxs_u16[:, ib + 8:ib + 16], vals[:, 8:16], met)
                nc.vector.match_replace(out=met, in_to_replace=vals[:, 8:16],
                                        in_values=met, imm_value=NEG)
                nc.vector.max(out=vals[:, 16:24], in_=met)
                nc.vector.max_index(idxs_u16[:, ib + 16:ib + 24], vals[:, 16:24], met)

            # wrapped idx tensor for dma_gather
            nc.vector.memset(widx, 0)
            iv = idxs_u16.bitcast(I16).rearrange("p (t c) -> p t c", c=24)
            wv = widx.rearrange("p (t k r) -> p t k r", k=K1, r=8)
            for rh in range(8):
                for t in range(T):
                    nc.sync.dma_start(out=wv[0:16, t, :, rh],
                                      in_=iv[16 * rh:16 * rh + 16, t, 0:K1])
            for g in range(1, 8):
                nc.sync.dma_start(out=widx[16 * g:16 * (g + 1), :],
                                  in_=widx[0:16, :])

        # ================= PHASE C: stage 0 =================
        with tc.tile_pool(name="pc0", bufs=2) as pc0, \
             tc.tile_pool(name="pc0_ps", bufs=2, space="PSUM") as pc0_ps:
            if skip_stage0:
                nc.vector.memset(x1T, 0.0)
                nc.vector.memset(S1_keep, 0.0)
                nc.vector.memset(ggf_keep, 0.0)
            for t in range(0 if skip_stage0 else T):
                rec0 = pc0.tile([128, K1, REC0], F32, tag="rec0")
                if gather_mode == "swdge":
                    nc.gpsimd.dma_gather(
                        out_ap=rec0, in_ap=table0.ap(),
                        idxs_ap=widx[:, S17 * t:S17 * (t + 1)],
                        num_idxs=K1 * 128, num_idxs_reg=K1 * 128,
                        elem_size=REC0, single_packet=False)
                elif gather_mode == "fake":
                    nc.sync.dma_start(
                        out=rec0,
                        in_=table0.ap()[0:K1 * 128, :]
                            .rearrange("(k p) c -> p k c", p=128))
                else:
                    nc.vector.memset(rec0, 0.0)

                # coordsT [85, 128]
                co5 = pc0.tile([128, K1 * 5], F32, tag="co5")
                nc.vector.tensor_copy(co5.rearrange("p (k c) -> p k c", c=5),
                                      rec0[:, :, 0:5])
                psc = pc0_ps.tile([85, 128], F32, tag="pssm")
                tr(psc, co5)
                coT = pc0.tile([85, 128], F32, tag="coT")
                nc.scalar.activation(coT, psc, AF.Copy)

                # S + stats [128, 101]
                pss = pc0_ps.tile([128, 101], F32, tag="pssm")
                nc.tensor.matmul(pss, coT, wsb["w_ss"], start=True, stop=True)
                sall = pc0.tile([128, 101], F32, tag="sall")
                nc.vector.tensor_copy(sall, pss)
                nc.vector.tensor_copy(S1_keep[:, 64 * t:64 * (t + 1)],
                                      sall[:, 37:101])

                # ggf
                mn = pc0.tile([128, 5], F32, tag="mn")
                nc.vector.tensor_scalar_mul(mn, sall[:, 32:37], 1.0 / K1)
                gg = pc0.tile([128, 8], F32, tag="gg")
                # gg cols: 0 cov, 1 vx, 2 vy, 3 tmp, 4 m, 5 pdist
                nc.vector.tensor_mul(gg[:, 3:4], mn[:, 0:1], mn[:, 1:2])
                nc.vector.tensor_sub(gg[:, 0:1], mn[:, 4:5], gg[:, 3:4])
                nc.vector.tensor_mul(gg[:, 3:4], mn[:, 0:1], mn[:, 0:1])
                nc.vector.tensor_sub(gg[:, 1:2], mn[:, 2:3], gg[:, 3:4])
                nc.vector.tensor_mul(gg[:, 3:4], mn[:, 1:2], mn[:, 1:2])
                nc.vector.tensor_sub(gg[:, 2:3], mn[:, 3:4], gg[:, 3:4])
                nc.vector.tensor_scalar_add(gg[:, 3:4], gg[:, 1:2], EPS)
                nc.vector.reciprocal(gg[:, 3:4], gg[:, 3:4])
                nc.vector.tensor_mul(gg[:, 4:5], gg[:, 0:1], gg[:, 3:4])
                nc.vector.tensor_mul(gg[:, 3:4], gg[:, 1:2], gg[:, 2:3])
                nc.vector.tensor_scalar_add(gg[:, 3:4], gg[:, 3:4], EPS)
                nc.scalar.activation(gg[:, 3:4], gg[:, 3:4], AF.Sqrt)
                nc.vector.reciprocal(gg[:, 3:4], gg[:, 3:4])
                nc.vector.tensor_mul(gg[:, 5:6], gg[:, 0:1], gg[:, 3:4])
                nc.vector.tensor_scalar(gg[:, 5:6], gg[:, 5:6], -1.0, 1.0,
                                        op0=mybir.AluOpType.mult,
                                        op1=mybir.AluOpType.add)
                nc.vector.tensor_copy(ggf_keep[:, 2 * t:2 * t + 2], gg[:, 4:6])

                # r0 = relu(A0slab + S0)
                r0 = pc0.tile([128, K1 * 32], F32, tag="r0")
                r0v = r0.rearrange("p (k c) -> p k c", c=32)
                nc.vector.tensor_add(r0v, rec0[:, :, 8:40],
                                     sall[:, 0:32].rearrange("p (u c) -> p u c", u=1).to_broadcast([128, K1, 32]))
                nc.vector.tensor_scalar_max(r0, r0, 0.0)

                # r0T [32, K1*128] via chunked 1-bank psum tiles
                r0T = pc0.tile([32, K1 * 128], F32, tag="r0T")
                for k in range(K1):
                    psk = pc0_ps.tile([128, 512], F32, tag="pst", bufs=6)
                    tr(psk[0:32, 0:128], r0[:, 32 * k:32 * (k + 1)])
                    nc.scalar.activation(r0T[:, 128 * k:128 * (k + 1)],
                                         psk[0:32, 0:128], AF.Copy)

                # s_rT [64, K1*128]
                srT = pc0.tile([64, K1 * 128], F32, tag="srT")
                for c in range(0, K1 * 128, 512):
                    ce = min(c + 512, K1 * 128)
                    psk = pc0_ps.tile([128, 512], F32, tag="pst", bufs=6)
                    nc.tensor.matmul(psk[0:64, 0:ce - c], wsb["w_sr0"],
                                     r0T[:, c:ce], start=True, stop=True)
                    nc.scalar.activation(srT[:, c:ce], psk[0:64, 0:ce - c],
                                         AF.Copy)

                # back-transpose chunks into SBUF slab + add P0 slab
                sl = pc0.tile([128, K1 * 64], F32, tag="slsb")
                for k in range(K1):
                    psk = pc0_ps.tile([128, 512], F32, tag="pst", bufs=6)
                    tr(psk[:, 0:64], srT[:, 128 * k:128 * (k + 1)])
                    nc.scalar.activation(sl[:, 64 * k:64 * (k + 1)],
                                         psk[:, 0:64], AF.Copy)
                ssb = pc0.tile([128, K1 * 64], F32, tag="ssb")
                nc.vector.tensor_add(ssb.rearrange("p (k c) -> p k c", c=64),
                                     sl.rearrange("p (k c) -> p k c", c=64),
                                     rec0[:, :, 72:136])
                esb = pc0.tile([128, K1 * 64], F32, tag="esb")
                nc.scalar.activation(esb, ssb, AF.Exp)
                zz = pc0.tile([128, K1], F32, tag="zz")
                nc.vector.tensor_reduce(zz, esb.rearrange("p (k c) -> p k c", c=64),
                                        axis=mybir.AxisListType.X,
                                        op=mybir.AluOpType.add)
                rz = pc0.tile([128, K1], F32, tag="rz")
                nc.vector.reciprocal(rz, zz)
                sm = pc0.tile([128, K1, 64], F32, tag="sm")
                nc.vector.tensor_mul(sm, esb.rearrange("p (k c) -> p k c", c=64),
                                     rz.rearrange("p (k u) -> p k u", u=1)
                                       .to_broadcast([128, K1, 64]))
                # attention
                pr = pc0.tile([128, K1, 32], F32, tag="pr")
                att = pc0.tile([128, 67], F32, tag="att")
                nc.gpsimd.tensor_mul(pr, sm[:, :, 0:32], rec0[:, :, 40:72])
                nc.vector.tensor_reduce(
                    att[:, 0:32], pr.rearrange("p k c -> p c k"),
                    axis=mybir.AxisListType.X, op=mybir.AluOpType.add)
                nc.vector.tensor_mul(pr, sm[:, :, 32:64],
                                     r0.rearrange("p (k c) -> p k c", c=32))
                nc.vector.tensor_reduce(
                    att[:, 32:64], pr.rearrange("p k c -> p c k"),
                    axis=mybir.AxisListType.X, op=mybir.AluOpType.add)
                nc.vector.tensor_copy(att[:, 64:66], gg[:, 4:6])
                nc.vector.memset(att[:, 66:67], 1.0)

                # x1 chunk = relu(w_f0^T att)
                psa = pc0_ps.tile([67, 128], F32, tag="pssm")
                tr(psa, att)
                attT = pc0.tile([67, 128], F32, tag="attT")
                nc.scalar.activation(attT, psa, AF.Copy)
                psx = pc0_ps.tile([64, 128], F32, tag="pssm")
                nc.tensor.matmul(psx, wsb["w_f0"], attT, start=True, stop=True)
                nc.scalar.activation(x1T[0:64, 128 * t:128 * (t + 1)], psx, AF.Relu)

            for c in range(0, R, min(N, 2048)):
                ce = min(c + min(N, 2048), R)
                nc.sync.dma_start(out=x1T[64:65, c:ce],
                                  in_=wt["ones_n"].ap()[:, 0:ce - c])
            # P1T + table1 records
            p1t = pc0.tile([128, R], F32, tag="p1t")
            for c in range(0, R, 512):
                ce = min(c + 512, R)
                ps = pc0_ps.tile([128, 512], F32, tag="pssm")
                nc.tensor.matmul(ps[:, 0:ce - c], wsb["w_p1"], x1T[:, c:ce],
                                 start=True, stop=True)
                nc.scalar.activation(p1t[:, c:ce], ps[:, 0:ce - c], AF.Copy)
            for t in range(T):
                ps = pc0_ps.tile([128, 192], F32, tag="pssm")
                sl = slice(128 * t, 128 * (t + 1))
                tr(ps[:, 0:64], x1T[0:64, sl])
                tr(ps[:, 64:192], p1t[:, sl])
                tbt1 = pc0.tile([128, 192], F32, tag="tbt1")
                nc.vector.tensor_copy(tbt1, ps)
                nc.sync.dma_start(out=table1_own.ap()[sl, 64:256], in_=tbt1)

        # ---- AllGather ----
        if no_cc:
            nc.sync.dma_start(out=table1.ap()[0:R, :], in_=table1_own.ap())
        elif CPB > 1:
            nc.gpsimd.collective_compute(
                "AllGather", mybir.AluOpType.bypass,
                replica_groups=groups,
                ins=[table1_own.ap()], outs=[table1.ap()])
        else:
            nc.sync.dma_start(out=table1.ap(), in_=table1_own.ap())

        # ================= PHASE D: stage 1 + final =================
        with tc.tile_pool(name="pd", bufs=2) as pd, \
             tc.tile_pool(name="pd_ps", bufs=2, space="PSUM") as pd_ps:
            if skip_stage1:
                zout = pd.tile([128, 128], F32, tag="zout")
                nc.vector.memset(zout, 0.0)
                for t in range(T):
                    nc.sync.dma_start(out=out.ap()[128 * t:128 * (t + 1), :],
                                      in_=zout)
            for t in range(0 if skip_stage1 else T):
                rec1 = pd.tile([128, K1, REC1], F32, tag="rec1")
                if gather_mode == "swdge":
                    nc.gpsimd.dma_gather(
                        out_ap=rec1, in_ap=table1.ap(),
                        idxs_ap=widx[:, S17 * t:S17 * (t + 1)],
                        num_idxs=K1 * 128, num_idxs_reg=K1 * 128,
                        elem_size=REC1, single_packet=False)
                elif gather_mode == "fake":
                    nc.sync.dma_start(
                        out=rec1,
                        in_=table1.ap()[0:K1 * 128, :]
                            .rearrange("(k p) c -> p k c", p=128))
                else:
                    nc.vector.memset(rec1, 0.0)

                r1 = pd.tile([128, K1 * 64], F32, tag="r1")
                nc.vector.tensor_add(
                    r1.rearrange("p (k c) -> p k c", c=64), rec1[:, :, 0:64],
                    S1_keep[:, 64 * t:64 * (t + 1)].rearrange("p (u c) -> p u c", u=1).to_broadcast([128, K1, 64]))
                nc.vector.tensor_scalar_max(r1, r1, 0.0)

                r1T = pd.tile([64, K1 * 128], F32, tag="r1T")
                for k in range(K1):
                    psk = pd_ps.tile([128, 512], F32, tag="pst1", bufs=6)
                    tr(psk[0:64, 0:128], r1[:, 64 * k:64 * (k + 1)])
                    nc.scalar.activation(r1T[:, 128 * k:128 * (k + 1)],
                                         psk[0:64, 0:128], AF.Copy)

                srT = pd.tile([128, K1 * 128], F32, tag="sr1T")
                for c in range(0, K1 * 128, 512):
                    ce = min(c + 512, K1 * 128)
                    psk = pd_ps.tile([128, 512], F32, tag="pst1", bufs=6)
                    nc.tensor.matmul(psk[:, 0:ce - c], wsb["w_sr1"],
                                     r1T[:, c:ce], start=True, stop=True)
                    nc.scalar.activation(srT[:, c:ce], psk[:, 0:ce - c], AF.Copy)

                sl = pd.tile([128, K1 * 128], F32, tag="slsb1")
                for k in range(K1):
                    psk = pd_ps.tile([128, 512], F32, tag="pst1", bufs=6)
                    tr(psk[:, 0:128], srT[:, 128 * k:128 * (k + 1)])
                    nc.scalar.activation(sl[:, 128 * k:128 * (k + 1)],
                                         psk[:, 0:128], AF.Copy)
                nc.vector.tensor_add(sl.rearrange("p (k c) -> p k c", c=128),
                                     sl.rearrange("p (k c) -> p k c", c=128),
                                     rec1[:, :, 128:256])
                esb = pd.tile([128, K1 * 128], F32, tag="esb1", bufs=1)
                nc.scalar.activation(esb, sl, AF.Exp)
                zz = pd.tile([128, K1], F32, tag="zz1")
                nc.vector.tensor_reduce(zz, esb.rearrange("p (k c) -> p k c", c=128),
                                        axis=mybir.AxisListType.X,
                                        op=mybir.AluOpType.add)
                rz = pd.tile([128, K1], F32, tag="rz1")
                nc.vector.reciprocal(rz, zz)
                sm = pd.tile([128, K1, 128], F32, tag="sm1", bufs=1)
                nc.vector.tensor_mul(sm, esb.rearrange("p (k c) -> p k c", c=128),
                                     rz.rearrange("p (k u) -> p k u", u=1)
                                       .to_broadcast([128, K1, 128]))
                pr = pd.tile([128, K1, 64], F32, tag="pr1")
                att = pd.tile([128, 128], F32, tag="att1")
                nc.gpsimd.tensor_mul(pr, sm[:, :, 0:64], rec1[:, :, 64:128])
                nc.vector.tensor_reduce(
                    att[:, 0:64], pr.rearrange("p k c -> p c k"),
                    axis=mybir.AxisListType.X, op=mybir.AluOpType.add)
                nc.vector.tensor_mul(pr, sm[:, :, 64:128],
                                     r1.rearrange("p (k c) -> p k c", c=64))
                nc.vector.tensor_reduce(
                    att[:, 64:128], pr.rearrange("p k c -> p c k"),
                    axis=mybir.AxisListType.X, op=mybir.AluOpType.add)

                # x2 = relu(Wf1a^T att1 + Wf1b^T [ggf; 1] + bf1)
                psa = pd_ps.tile([128, 128], F32, tag="pssm1")
                tr(psa, att)
                attT = pd.tile([128, 128], F32, tag="attT1")
                nc.scalar.activation(attT, psa, AF.Copy)
                gco = pd.tile([128, 3], F32, tag="gco")
                nc.vector.tensor_copy(gco[:, 0:2], ggf_keep[:, 2 * t:2 * t + 2])
                nc.vector.memset(gco[:, 2:3], 1.0)
                psg = pd_ps.tile([3, 128], F32, tag="pssm1")
                tr(psg, gco)
                gcoT = pd.tile([3, 128], F32, tag="gcoT")
                nc.scalar.activation(gcoT, psg, AF.Copy)
                psx = pd_ps.tile([128, 128], F32, tag="pssm1")
                nc.tensor.matmul(psx, wsb["w_f1a"], attT, start=True, stop=False)
                nc.tensor.matmul(psx, wsb["w_f1b"], gcoT, start=False, stop=True)
                x2T = pd.tile([128, 128], F32, tag="x2T")
                nc.scalar.activation(x2T, psx, AF.Relu)

                # x3 = relu(W1^T x2 + b1); out = leaky(x3 + y, 0.2)
                pso = pd_ps.tile([128, 128], F32, tag="pssm1")
                nc.tensor.matmul(pso, wsb["w_out"], x2T, start=True, stop=True)
                x3T = pd.tile([128, 128], F32, tag="x3T")
                nc.scalar.activation(x3T, pso, AF.Relu, bias=wsb["b_out"][:, 0:1])
                nc.vector.tensor_add(x3T, x3T, y_resT[:, 128 * t:128 * (t + 1)])
                pos = pd.tile([128, 128], F32, tag="pos")
                nc.vector.tensor_scalar_max(pos, x3T, 0.0)
                nc.vector.tensor_scalar(x3T, x3T, 0.0, 0.2,
                                        op0=mybir.AluOpType.min,
                                        op1=mybir.AluOpType.mult)
                nc.vector.tensor_add(x3T, x3T, pos)

                psf = pd_ps.tile([128, 128], F32, tag="pssm1")
                tr(psf, x3T)
                orow = pd.tile([128, 128], F32, tag="orow")
                nc.scalar.activation(orow, psf, AF.Copy)
                nc.sync.dma_start(out=out.ap()[128 * t:128 * (t + 1), :], in_=orow)

        pp.release()

    nc.compile()
    return nc


_CACHE = {}


def _get_module(N, CPB, n_cores):
    key = (N, CPB, n_cores)
    if key not in _CACHE:
        _CACHE[key] = build_module(N, CPB, n_cores)
    return _CACHE[key]


def kernel(**inputs):
    pc = np.asarray(inputs["pc"])
    feats = np.asarray(inputs["feats"])
    Bv, N, _ = pc.shape
    n_cores = 8
    CPB = n_cores // Bv
    R = N // CPB
    nc = _get_module(N, CPB, n_cores)
    wts = host_weights(inputs, N)
    in_maps = []
    for c in range(n_cores):
        b = c // CPB
        s = (c % CPB) * R
        m = {"pc_b": np.ascontiguousarray(pc[b]),
             "feats_b": np.ascontiguousarray(feats[b]),
             "pc_own": np.ascontiguousarray(pc[b, s:s + R]),
             "feats_own": np.ascontiguousarray(feats[b, s:s + R])}
        m.update(wts)
        in_maps.append(m)
    res = run_bass_kernel_spmd(nc, in_maps, list(range(n_cores)))
    out = np.empty((Bv, N, 128), np.float32)
    for c in range(n_cores):
        b = c // CPB
        s = (c % CPB) * R
        out[b, s:s + R] = res.results[c]["out"]
    return out



# revision 34
# speedup vs baseline: 21.6452x; 9.7068x over previous
"""Bass/Trainium2 kernel for nn_DilatedResBlock (RandLA-Net style block).

Sharding: batch B=2 x 4-way split of N=8192 points -> 8 cores.
Each core: KNN top-17 for its 2048 rows (full-batch candidate scan via PE
matmul metric + VectorE max8/match_replace rounds), geometry encoding via
algebraic fold (per-point A-tables + per-row S-matmul), two attentive
pooling stages with per-neighbor record gathers (SWDGE dma_gather), one
AllGather of stage-0 outputs across the 4 cores of a batch.
"""

import numpy as np

import concourse.bass as bass
import concourse.mybir as mybir
from concourse import bacc
from concourse.bass_utils import run_bass_kernel_spmd
from concourse.tile import TileContext

F32 = mybir.dt.float32
BF16 = mybir.dt.bfloat16
U16 = mybir.dt.uint16
I16 = mybir.dt.int16
I32 = mybir.dt.int32

B, DIMS, K1 = 2, 2, 17
IN_UNITS, UNITS = 64, 128
EPS = 1e-8
NEG = -3.0e38

AF = mybir.ActivationFunctionType


def host_weights(w, N):
    """Host-side rearrangements of the given weights (no data-dependent work)."""
    f = np.float32
    Wl0, bl0 = w["Wl0"], w["bl0"]
    Wl1, bl1 = w["Wl1"], w["bl1"]
    # A-fold: r0_pre[n,j] = [x_j,y_j,r_j,1] @ w_a + S[n]
    def afold(Wl, bl):
        xco = Wl[0] - Wl[2::3].sum(0)
        yco = Wl[1] - Wl[3::3].sum(0)
        rco = Wl[4::3].sum(0)
        return np.stack([xco, yco, rco, bl]).astype(f)

    w_a0 = afold(Wl0, bl0)          # [4, 32]
    w_a1 = afold(Wl1, bl1)          # [4, 64]

    # S + stats matmul weights: coords5 channel (k, j), j in {x,y,x2,y2,xy}
    w_ss = np.zeros((5 * K1, 32 + 5 + 64), f)
    for k in range(K1):
        w_ss[5 * k + 0, 0:32] = Wl0[2 + 3 * k]
        w_ss[5 * k + 1, 0:32] = Wl0[3 + 3 * k]
        w_ss[5 * k + 0, 32] = 1.0
        w_ss[5 * k + 1, 33] = 1.0
        w_ss[5 * k + 2, 34] = 1.0
        w_ss[5 * k + 3, 35] = 1.0
        w_ss[5 * k + 4, 36] = 1.0
        w_ss[5 * k + 0, 37:101] = Wl1[2 + 3 * k]
        w_ss[5 * k + 1, 37:101] = Wl1[3 + 3 * k]

    vs = lambda W, b: np.vstack([W, b[None, :]]).astype(f)
    w_sr0 = w["Ws0"][32:64].astype(f)
    d2 = np.zeros((64, 128), f)
    d2[0:32, 0:64] = w_sr0
    d2[32:64, 64:128] = w_sr0
    z64 = np.zeros((64, 128), f)
    w_sr0e = np.vstack([d2, z64])                 # contract rows 0:64
    w_sr0o = np.vstack([z64, d2])                 # contract rows 64:128
    w_sr1 = w["Ws1"][64:128].astype(f)
    w_sr1e = np.vstack([w_sr1, z64])
    w_sr1o = np.vstack([z64, w_sr1])
    return {
        "w_sr0e": w_sr0e, "w_sr0o": w_sr0o,
        "w_sr1e": w_sr1e, "w_sr1o": w_sr1o,
        "w_a0": w_a0, "w_a1": w_a1, "w_ss": w_ss.astype(f),
        "w_mlp0": vs(w["W0"], w["b0"]),          # [65, 32]
        "w_res": vs(w["W_res"], w["b_res"]),     # [65, 128]
        "w_p0": vs(w["Ws0"][0:32], w["bs0"]),    # [33, 64]
        "w_sr0": w["Ws0"][32:64].astype(f),      # [32, 64]
        "w_p1": vs(w["Ws1"][0:64], w["bs1"]),    # [65, 128]
        "w_sr1": w["Ws1"][64:128].astype(f),     # [64, 128]
        "w_f0": vs(w["Wf0"], w["bf0"]),          # [67, 64]
        "w_f1a": w["Wf1"][0:128].astype(f),      # [128, 128]
        "w_f1b": vs(w["Wf1"][128:130], w["bf1"]),# [3, 128]
        "w_out": w["W1"].astype(f),              # [128, 128]
        "b_out": w["b1"].reshape(128, 1).astype(f),
        "ident": np.eye(128, dtype=f),
        "ones_n": np.ones((1, min(N, 2048)), f),
    }


WNAMES = ["w_a0", "w_a1", "w_ss", "w_mlp0", "w_res", "w_p0", "w_sr0",
          "w_sr0e", "w_sr0o", "w_sr1e", "w_sr1o", "w_p1", "w_sr1", "w_f0",
          "w_f1a", "w_f1b", "w_out", "b_out", "ident", "ones_n"]
WSHAPES = None  # filled in build


def build_module(N, CPB, n_cores, no_cc=False, skip_topk=False,
                 gather_mode="swdge", skip_stage0=False, skip_stage1=False,
                 topk_mode="f32"):
    """Build the SPMD Bass module. R = N // CPB own rows per core."""
    R = N // CPB
    T = R // 128      # own row tiles
    PT = N // 128     # full-batch point tiles
    REC0 = 192
    REC1 = 256
    S17 = 136         # wrapped idx cols per tile (17*8)

    nc = bacc.Bacc("TRN2", target_bir_lowering=False, debug=False,
                   num_devices=n_cores)

    # --- I/O ---
    pc_b = nc.dram_tensor("pc_b", [N, 2], F32, kind="ExternalInput")
    feats_b = nc.dram_tensor("feats_b", [N, 64], F32, kind="ExternalInput")
    pc_own = nc.dram_tensor("pc_own", [R, 2], F32, kind="ExternalInput")
    feats_own = nc.dram_tensor("feats_own", [R, 64], F32, kind="ExternalInput")
    wt = {}
    shapes = {"w_a0": [4, 32], "w_a1": [4, 64], "w_ss": [85, 101],
              "w_mlp0": [65, 32], "w_res": [65, 128], "w_p0": [33, 64],
              "w_sr0": [32, 64], "w_sr0e": [128, 128], "w_sr0o": [128, 128],
              "w_sr1e": [128, 128], "w_sr1o": [128, 128],
              "w_p1": [65, 128],
              "w_sr1": [64, 128], "w_f0": [67, 64], "w_f1a": [128, 128],
              "w_f1b": [3, 128], "w_out": [128, 128], "b_out": [128, 1],
              "ident": [128, 128], "ones_n": [1, min(N, 2048)]}
    for nm in WNAMES:
        wt[nm] = nc.dram_tensor(nm, shapes[nm], F32, kind="ExternalInput")
    out = nc.dram_tensor("out", [R, 128], F32, kind="ExternalOutput")

    table0 = nc.dram_tensor("table0", [N, REC0], F32)
    table1_own = nc.dram_tensor("table1_own", [R, REC1], F32)
    table1 = nc.dram_tensor("table1", [N, REC1], F32)

    groups = [[g * CPB + j for j in range(CPB)] for g in range(n_cores // CPB)]

    with TileContext(nc) as tc:
        # ---- persistent pool (lives whole kernel) ----
        pp = tc.alloc_tile_pool(name="persist", bufs=1)
        wsb = {nm: pp.tile(shapes[nm], F32, tag=nm, name=nm) for nm in WNAMES}
        for nm in WNAMES:
            nc.sync.dma_start(out=wsb[nm], in_=wt[nm].ap())
        ident = wsb["ident"]

        def tr(out_ap, in_ap):
            k = in_ap.shape[0]
            nc.tensor.transpose(out_ap, in_ap, ident[0:k, 0:k])

        rhs3 = pp.tile([3, N], F32, tag="rhs3")
        pcT_own = pp.tile([2, R], F32, tag="pcT_own")
        y_resT = pp.tile([128, R], F32, tag="y_resT")
        S1_keep = pp.tile([128, T * 64], F32, tag="S1_keep")
        ggf_keep = pp.tile([128, T * 2], F32, tag="ggf_keep")
        x1T = pp.tile([65, R], F32, tag="x1T")
        idxs_u16 = pp.tile([128, T * 24], U16, tag="idxs_u16")
        widx = pp.tile([128, T * S17], I16, tag="widx")

        # ================= PHASE A: tables =================
        OC = min(N, 2048)

        def ones_dma(dst_ap, length):
            for c in range(0, length, OC):
                ce = min(c + OC, length)
                nc.sync.dma_start(out=dst_ap[:, c:ce],
                                  in_=wt["ones_n"].ap()[:, 0:ce - c])

        # ---- A-geom scope ----
        with tc.tile_pool(name="pag", bufs=1) as pa, \
             tc.tile_pool(name="pag_ch", bufs=2) as pch, \
             tc.tile_pool(name="pa_ps", bufs=3, space="PSUM") as pa_ps:
            pc_rm = pa.tile([128, PT * 2], F32, tag="pc_rm")
            nc.sync.dma_start(out=pc_rm.rearrange("p (t c) -> p t c", c=2),
                              in_=pc_b.ap().rearrange("(t p) c -> p t c", p=128))
            pco_rm = pa.tile([128, T * 2], F32, tag="pco_rm")
            nc.sync.dma_start(out=pco_rm.rearrange("p (t c) -> p t c", c=2),
                              in_=pc_own.ap().rearrange("(t p) c -> p t c", p=128))

            # cA = [x; y; r; 1] channel-major, full batch
            cA = pa.tile([4, N], F32, tag="cA")
            for t in range(PT):
                ps = pa_ps.tile([2, 128], F32, tag="ps")
                tr(ps, pc_rm[:, 2 * t:2 * t + 2])
                nc.scalar.activation(cA[0:2, 128 * t:128 * (t + 1)], ps, AF.Copy)
            for t in range(T):
                ps = pa_ps.tile([2, 128], F32, tag="ps")
                tr(ps, pco_rm[:, 2 * t:2 * t + 2])
                nc.scalar.activation(pcT_own[:, 128 * t:128 * (t + 1)], ps, AF.Copy)
            nc.vector.tensor_copy(rhs3[0:2, :], cA[0:2, :])
            ones21 = pa.tile([2, 1], F32, tag="ones21")
            nc.vector.memset(ones21, 1.0)
            # sq and r rows, chunked; -sq goes into rhs3[2:3] via metric sign trick
            for c in range(0, N, 512):
                ce = min(c + 512, N)
                sqch = pch.tile([2, 512], F32, tag="sqch")
                nc.vector.tensor_mul(sqch[:, 0:ce - c], cA[0:2, c:ce],
                                     cA[0:2, c:ce])
                ps = pa_ps.tile([1, 512], F32, tag="ps")
                nc.tensor.matmul(ps[:, 0:ce - c], ones21, sqch[:, 0:ce - c],
                                 start=True, stop=True)
                sqv = pch.tile([1, 512], F32, tag="sqv")
                nc.scalar.activation(sqv[:, 0:ce - c], ps[:, 0:ce - c], AF.Copy)
                nc.sync.dma_start(out=rhs3[2:3, c:ce], in_=sqv[:, 0:ce - c])
                rv = pch.tile([1, 512], F32, tag="rv")
                nc.scalar.activation(rv[:, 0:ce - c], ps[:, 0:ce - c], AF.Sqrt)
                nc.sync.dma_start(out=cA[2:3, c:ce], in_=rv[:, 0:ce - c])
            ones_dma(cA[3:4, :], N)

            cAo = pa.tile([4, R], F32, tag="cAo")
            nc.vector.tensor_copy(cAo[0:2, :], pcT_own)
            for c in range(0, R, 512):
                ce = min(c + 512, R)
                sqch = pch.tile([2, 512], F32, tag="sqch")
                nc.vector.tensor_mul(sqch[:, 0:ce - c], pcT_own[:, c:ce],
                                     pcT_own[:, c:ce])
                ps = pa_ps.tile([1, 512], F32, tag="ps")
                nc.tensor.matmul(ps[:, 0:ce - c], ones21, sqch[:, 0:ce - c],
                                 start=True, stop=True)
                rv = pch.tile([1, 512], F32, tag="rv")
                nc.scalar.activation(rv[:, 0:ce - c], ps[:, 0:ce - c], AF.Sqrt)
                nc.sync.dma_start(out=cAo[2:3, c:ce], in_=rv[:, 0:ce - c])
            ones_dma(cAo[3:4, :], R)

            # A0T full batch / A1T own
            a0t = pa.tile([32, N], F32, tag="a0t")
            a1to = pa.tile([64, R], F32, tag="a1to")
            for c in range(0, N, 512):
                ce = min(c + 512, N)
                ps = pa_ps.tile([32, 512], F32, tag="ps")
                nc.tensor.matmul(ps[:, 0:ce - c], wsb["w_a0"], cA[:, c:ce],
                                 start=True, stop=True)
                nc.scalar.activation(a0t[:, c:ce], ps[:, 0:ce - c], AF.Copy)
            for c in range(0, R, 512):
                ce = min(c + 512, R)
                ps = pa_ps.tile([64, 512], F32, tag="ps")
                nc.tensor.matmul(ps[:, 0:ce - c], wsb["w_a1"], cAo[:, c:ce],
                                 start=True, stop=True)
                nc.scalar.activation(a1to[:, c:ce], ps[:, 0:ce - c], AF.Copy)

            # table0 geometry columns: [x,y,x2,y2,xy,0,0,0 | A0(32)]
            tb0s = pa.tile([128, PT * 8], F32, tag="tb0s")
            nc.vector.memset(tb0s, 0.0)
            tsv = tb0s.rearrange("p (t c) -> p t c", c=8)
            prv = pc_rm.rearrange("p (t c) -> p t c", c=2)
            xv = prv[:, :, 0:1]
            yv = prv[:, :, 1:2]
            nc.vector.tensor_copy(tsv[:, :, 0:2], prv)
            nc.vector.tensor_mul(tsv[:, :, 2:3], xv, xv)
            nc.vector.tensor_mul(tsv[:, :, 3:4], yv, yv)
            nc.vector.tensor_mul(tsv[:, :, 4:5], xv, yv)
            nc.sync.dma_start(
                out=table0.ap().rearrange("(t p) c -> p t c", p=128)[:, :, 0:8],
                in_=tsv)
            zpad = pa.tile([128, 64], F32, tag="zpad")
            nc.vector.memset(zpad, 0.0)
            for t in range(PT):
                ps = pa_ps.tile([128, 32], F32, tag="psr")
                tr(ps, a0t[:, 128 * t:128 * (t + 1)])
                tbt = pch.tile([128, 32], F32, tag="tbt")
                nc.vector.tensor_copy(tbt, ps)
                nc.sync.dma_start(out=table0.ap()[128 * t:128 * (t + 1), 8:40],
                                  in_=tbt)
                nc.sync.dma_start(out=table0.ap()[128 * t:128 * (t + 1), 136:192],
                                  in_=zpad[:, 0:56])
            for t in range(T):
                ps = pa_ps.tile([128, 64], F32, tag="psr")
                tr(ps, a1to[:, 128 * t:128 * (t + 1)])
                tbt = pch.tile([128, 64], F32, tag="tbt1")
                nc.vector.tensor_copy(tbt, ps)
                nc.sync.dma_start(out=table1_own.ap()[128 * t:128 * (t + 1), 0:64],
                                  in_=tbt)

        # ---- A-feats scope ----
        with tc.tile_pool(name="paf", bufs=1) as pa, \
             tc.tile_pool(name="paf_ch", bufs=2) as pch, \
             tc.tile_pool(name="paf_ps", bufs=2, space="PSUM") as pa_ps:
            ftp = pa.tile([65, N], F32, tag="ftp")
            for t in range(PT):
                frt = pch.tile([128, 64], F32, tag="frt")
                nc.sync.dma_start(out=frt,
                                  in_=feats_b.ap()[128 * t:128 * (t + 1), :])
                ps = pa_ps.tile([64, 128], F32, tag="ps")
                tr(ps, frt)
                nc.scalar.activation(ftp[0:64, 128 * t:128 * (t + 1)], ps, AF.Copy)
            ones_dma(ftp[64:65, :], N)
            ftpo = pa.tile([65, R], F32, tag="ftpo")
            for t in range(T):
                frt = pch.tile([128, 64], F32, tag="frt")
                nc.sync.dma_start(out=frt,
                                  in_=feats_own.ap()[128 * t:128 * (t + 1), :])
                ps = pa_ps.tile([64, 128], F32, tag="ps")
                tr(ps, frt)
                nc.scalar.activation(ftpo[0:64, 128 * t:128 * (t + 1)], ps, AF.Copy)
            ones_dma(ftpo[64:65, :], R)

            for c in range(0, R, 512):
                ce = min(c + 512, R)
                ps = pa_ps.tile([128, 512], F32, tag="psm")
                nc.tensor.matmul(ps[:, 0:ce - c], wsb["w_res"], ftpo[:, c:ce],
                                 start=True, stop=True)
                nc.scalar.activation(y_resT[:, c:ce], ps[:, 0:ce - c], AF.Relu)

            x0tp = pa.tile([33, N], F32, tag="x0tp")
            for c in range(0, N, 512):
                ce = min(c + 512, N)
                ps = pa_ps.tile([32, 512], F32, tag="psm")
                nc.tensor.matmul(ps[:, 0:ce - c], wsb["w_mlp0"], ftp[:, c:ce],
                                 start=True, stop=True)
                nc.scalar.activation(x0tp[0:32, c:ce], ps[:, 0:ce - c], AF.Relu)
            ones_dma(x0tp[32:33, :], N)

            p0t = pa.tile([64, N], F32, tag="p0t")
            for c in range(0, N, 512):
                ce = min(c + 512, N)
                ps = pa_ps.tile([64, 512], F32, tag="psm")
                nc.tensor.matmul(ps[:, 0:ce - c], wsb["w_p0"], x0tp[:, c:ce],
                                 start=True, stop=True)
                nc.scalar.activation(p0t[:, c:ce], ps[:, 0:ce - c], AF.Copy)

            # table0 feature columns: x0 (40:72), P0 (72:136)
            for t in range(PT):
                ps = pa_ps.tile([128, 96], F32, tag="psr")
                sl = slice(128 * t, 128 * (t + 1))
                tr(ps[:, 0:32], x0tp[0:32, sl])
                tr(ps[:, 32:96], p0t[:, sl])
                tbt = pch.tile([128, 96], F32, tag="tbt")
                nc.vector.tensor_copy(tbt, ps)
                nc.sync.dma_start(out=table0.ap()[sl, 40:136], in_=tbt)

        # ================= PHASE B: KNN =================
        # i32 mode: composite key int32(met * SQ * 8192) + col_idx. Top-24 by
        # 3x(max8 + match_replace) on distinct int keys; idx = key & 8191.
        # Saves the three max_index scans on DVE.
        SQ = 2048.0
        with tc.tile_pool(name="pb", bufs=2) as pb, \
             tc.tile_pool(name="pb_ps", bufs=2, space="PSUM") as pb_ps:
            if topk_mode == "i32":
                iota = pb.tile([128, N], I32, tag="iota", bufs=1)
                nc.gpsimd.iota(iota, [[1, N]], base=0, channel_multiplier=0)
            idxs_i32 = pp.tile([128, T * 24], I32, tag="idxs_i32")
            for t in range(T):
                lhs3 = pb.tile([3, 128], F32, tag="lhs3")
                nc.vector.memset(lhs3, -1.0)
                nc.vector.tensor_scalar_mul(lhs3[0:2, :],
                                            pcT_own[:, 128 * t:128 * (t + 1)], 2.0)
                CW = min(N, 2048)
                ib = 24 * t
                if skip_topk:
                    nc.vector.memset(idxs_u16[:, ib:ib + 24], 0)
                    continue
                if topk_mode == "i32":
                    meti = pb.tile([128, N], I32, tag="meti")
                    for c in range(0, N, CW):
                        ps = pb_ps.tile([128, CW], F32, tag="ps_met")
                        for c2 in range(0, CW, 512):
                            nc.tensor.matmul(ps[:, c2:c2 + 512], lhs3,
                                             rhs3[:, c + c2:c + c2 + 512],
                                             start=True, stop=True)
                        nc.scalar.activation(meti[:, c:c + CW], ps, AF.Copy,
                                             scale=SQ * 8192.0)
                    nc.gpsimd.tensor_add(meti, meti, iota)
                    vals = pb.tile([128, 24], I32, tag="valsi")
                    nc.vector.max(out=vals[:, 0:8], in_=meti)
                    nc.vector.match_replace(out=meti, in_to_replace=vals[:, 0:8],
                                            in_values=meti,
                                            imm_value=-2147483648.0)
                    nc.vector.max(out=vals[:, 8:16], in_=meti)
                    nc.vector.match_replace(out=meti, in_to_replace=vals[:, 8:16],
                                            in_values=meti,
                                            imm_value=-2147483648.0)
                    nc.vector.max(out=vals[:, 16:24], in_=meti)
                    nc.vector.tensor_scalar(idxs_i32[:, ib:ib + 24], vals, 8191, 0,
                                            op0=mybir.AluOpType.bitwise_and,
                                            op1=mybir.AluOpType.bitwise_or)
                else:
                    met = pb.tile([128, N], F32, tag="met")
                    for c in range(0, N, CW):
                        ps = pb_ps.tile([128, CW], F32, tag="ps_met")
                        for c2 in range(0, CW, 512):
                            nc.tensor.matmul(ps[:, c2:c2 + 512], lhs3,
                                             rhs3[:, c + c2:c + c2 + 512],
                                             start=True, stop=True)
                        nc.scalar.activation(met[:, c:c + CW], ps, AF.Copy)
                    vals = pb.tile([128, 24], F32, tag="vals")
                    nc.vector.max(out=vals[:, 0:8], in_=met)
                    nc.vector.max_index(idxs_u16[:, ib:ib + 8], vals[:, 0:8], met)
                    nc.vector.match_replace(out=met, in_to_replace=vals[:, 0:8],
                                            in_values=met, imm_value=NEG)
                    nc.vector.max(out=vals[:, 8:16], in_=met)
                    nc.vector.max_index(idxs_u16[:, ib + 8:ib + 16], vals[:, 8:16],
                                        met)
                    nc.vector.match_replace(out=met, in_to_replace=vals[:, 8:16],
                                            in_values=met, imm_value=NEG)
                    nc.vector.max(out=vals[:, 16:24], in_=met)
                    nc.vector.max_index(idxs_u16[:, ib + 16:ib + 24],
                                        vals[:, 16:24], met)

            # wrapped idx tensor for dma_gather
            nc.vector.memset(widx, 0)
            if topk_mode == "i32":
                # int32 idx values fit in the low 16 bits: read them via a
                # stride-2 int16 view of the i32 tile.
                iv = idxs_i32.bitcast(I16) \
                    .rearrange("p (t c h) -> p t c h", c=24, h=2)[:, :, :, 0]
            else:
                iv = idxs_u16.bitcast(I16).rearrange("p (t c) -> p t c", c=24)
            wv = widx.rearrange("p (t k r) -> p t k r", k=K1, r=8)
            for rh in range(8):
                for t in range(T):
                    nc.sync.dma_start(out=wv[0:16, t, :, rh],
                                      in_=iv[16 * rh:16 * rh + 16, t, 0:K1])
            for g in range(1, 8):
                nc.sync.dma_start(out=widx[16 * g:16 * (g + 1), :],
                                  in_=widx[0:16, :])

        # ================= PHASE C: stage 0 =================
        with tc.tile_pool(name="pc0", bufs=2) as pc0, \
             tc.tile_pool(name="pc0_ps", bufs=2, space="PSUM") as pc0_ps:
            if skip_stage0:
                nc.vector.memset(x1T, 0.0)
                nc.vector.memset(S1_keep, 0.0)
                nc.vector.memset(ggf_keep, 0.0)
            for t in range(0 if skip_stage0 else T):
                rec0 = pc0.tile([128, K1, REC0], F32, tag="rec0")
                if gather_mode == "swdge":
                    nc.gpsimd.dma_gather(
                        out_ap=rec0, in_ap=table0.ap(),
                        idxs_ap=widx[:, S17 * t:S17 * (t + 1)],
                        num_idxs=K1 * 128, num_idxs_reg=K1 * 128,
                        elem_size=REC0, single_packet=False)
                elif gather_mode == "fake":
                    nc.sync.dma_start(
                        out=rec0,
                        in_=table0.ap()[0:K1 * 128, :]
                            .rearrange("(k p) c -> p k c", p=128))
                else:
                    nc.vector.memset(rec0, 0.0)

                # coordsT [85, 128]
                co5 = pc0.tile([128, K1 * 5], F32, tag="co5")
                nc.vector.tensor_copy(co5.rearrange("p (k c) -> p k c", c=5),
                                      rec0[:, :, 0:5])
                psc = pc0_ps.tile([85, 128], F32, tag="pssm")
                tr(psc, co5)
                coT = pc0.tile([85, 128], F32, tag="coT")
                nc.scalar.activation(coT, psc, AF.Copy)

                # S + stats [128, 101]
                pss = pc0_ps.tile([128, 101], F32, tag="pssm")
                nc.tensor.matmul(pss, coT, wsb["w_ss"], start=True, stop=True)
                sall = pc0.tile([128, 101], F32, tag="sall")
                nc.vector.tensor_copy(sall, pss)
                nc.vector.tensor_copy(S1_keep[:, 64 * t:64 * (t + 1)],
                                      sall[:, 37:101])

                # ggf
                mn = pc0.tile([128, 5], F32, tag="mn")
                nc.vector.tensor_scalar_mul(mn, sall[:, 32:37], 1.0 / K1)
                gg = pc0.tile([128, 8], F32, tag="gg")
                # gg cols: 0 cov, 1 vx, 2 vy, 3 tmp, 4 m, 5 pdist
                nc.vector.tensor_mul(gg[:, 3:4], mn[:, 0:1], mn[:, 1:2])
                nc.vector.tensor_sub(gg[:, 0:1], mn[:, 4:5], gg[:, 3:4])
                nc.vector.tensor_mul(gg[:, 3:4], mn[:, 0:1], mn[:, 0:1])
                nc.vector.tensor_sub(gg[:, 1:2], mn[:, 2:3], gg[:, 3:4])
                nc.vector.tensor_mul(gg[:, 3:4], mn[:, 1:2], mn[:, 1:2])
                nc.vector.tensor_sub(gg[:, 2:3], mn[:, 3:4], gg[:, 3:4])
                nc.vector.tensor_scalar_add(gg[:, 3:4], gg[:, 1:2], EPS)
                nc.vector.reciprocal(gg[:, 3:4], gg[:, 3:4])
                nc.vector.tensor_mul(gg[:, 4:5], gg[:, 0:1], gg[:, 3:4])
                nc.vector.tensor_mul(gg[:, 3:4], gg[:, 1:2], gg[:, 2:3])
                nc.vector.tensor_scalar_add(gg[:, 3:4], gg[:, 3:4], EPS)
                nc.scalar.activation(gg[:, 3:4], gg[:, 3:4], AF.Sqrt)
                nc.vector.reciprocal(gg[:, 3:4], gg[:, 3:4])
                nc.vector.tensor_mul(gg[:, 5:6], gg[:, 0:1], gg[:, 3:4])
                nc.vector.tensor_scalar(gg[:, 5:6], gg[:, 5:6], -1.0, 1.0,
                                        op0=mybir.AluOpType.mult,
                                        op1=mybir.AluOpType.add)
                nc.vector.tensor_copy(ggf_keep[:, 2 * t:2 * t + 2], gg[:, 4:6])

                # r0 = relu(A0slab + S0)
                r0 = pc0.tile([128, K1 * 32], F32, tag="r0")
                r0v = r0.rearrange("p (k c) -> p k c", c=32)
                nc.vector.tensor_add(r0v, rec0[:, :, 8:40],
                                     sall[:, 0:32].rearrange("p (u c) -> p u c", u=1).to_broadcast([128, K1, 32]))
                nc.vector.tensor_scalar_max(r0, r0, 0.0)

                # r0T4 [128, 5*128]: 4-neighbor-packed transposes of r0
                r0T4 = pc0.tile([128, 5 * 128], F32, tag="r0T4")
                psk = pc0_ps.tile([128, 512], F32, tag="pst", bufs=4)
                for b in range(4):
                    tr(psk[:, 128 * b:128 * (b + 1)],
                       r0[:, 128 * b:128 * (b + 1)])
                nc.scalar.activation(r0T4[:, 0:512], psk, AF.Copy)
                psk = pc0_ps.tile([128, 512], F32, tag="pst", bufs=4)
                tr(psk[0:32, 0:128], r0[:, 512:544])
                nc.scalar.activation(r0T4[0:32, 512:640], psk[0:32, 0:128],
                                     AF.Copy)

                # srT2 [128, 9*128]: 2-neighbor-packed w_sr0 products
                srT2 = pc0.tile([128, 9 * 128], F32, tag="srT2")
                for pg in range(3):  # psum groups of up to 4 pair-blocks
                    js = range(4 * pg, min(4 * pg + 4, 9))
                    psk = pc0_ps.tile([128, 512], F32, tag="pst", bufs=4)
                    for j in js:
                        o = 128 * (j - 4 * pg)
                        if j < 8:
                            wname = "w_sr0o" if j % 2 else "w_sr0e"
                            nc.tensor.matmul(
                                psk[:, o:o + 128], wsb[wname],
                                r0T4[:, 128 * (j // 2):128 * (j // 2) + 128],
                                start=True, stop=True)
                        else:
                            nc.tensor.matmul(
                                psk[0:64, o:o + 128], wsb["w_sr0"],
                                r0T4[0:32, 512:640], start=True, stop=True)
                    if js.stop == 9:
                        w = 128 * (len(js) - 1)
                        if w:
                            nc.scalar.activation(
                                srT2[:, 512 * pg:512 * pg + w],
                                psk[:, 0:w], AF.Copy)
                        nc.scalar.activation(
                            srT2[0:64, 512 * pg + w:512 * pg + w + 128],
                            psk[0:64, w:w + 128], AF.Copy)
                    else:
                        w = 128 * len(js)
                        nc.scalar.activation(srT2[:, 512 * pg:512 * pg + w],
                                             psk[:, 0:w], AF.Copy)

                # back-transpose pair-blocks into sl [128, K1*64]
                sl = pc0.tile([128, K1 * 64], F32, tag="slsb")
                for pg in range(3):
                    js = range(4 * pg, min(4 * pg + 4, 9))
                    psk = pc0_ps.tile([128, 512], F32, tag="pst", bufs=4)
                    for j in js:
                        o = 128 * (j - 4 * pg)
                        if j < 8:
                            tr(psk[:, o:o + 128],
                               srT2[:, 128 * j:128 * (j + 1)])
                        else:
                            tr(psk[:, o:o + 64], srT2[0:64, 1024:1152])
                    w = 128 * len(js)
                    if js.stop == 9:
                        w = 128 * (len(js) - 1) + 64
                    nc.scalar.activation(sl[:, 128 * 4 * pg:128 * 4 * pg + w],
                                         psk[:, 0:w], AF.Copy)
                ssb = pc0.tile([128, K1 * 64], F32, tag="ssb")
                nc.vector.tensor_add(ssb.rearrange("p (k c) -> p k c", c=64),
                                     sl.rearrange("p (k c) -> p k c", c=64),
                                     rec0[:, :, 72:136])
                esb = pc0.tile([128, K1 * 64], F32, tag="esb")
                nc.scalar.activation(esb, ssb, AF.Exp)
                zz = pc0.tile([128, K1], F32, tag="zz")
                nc.vector.tensor_reduce(zz, esb.rearrange("p (k c) -> p k c", c=64),
                                        axis=mybir.AxisListType.X,
                                        op=mybir.AluOpType.add)
                rz = pc0.tile([128, K1], F32, tag="rz")
                nc.vector.reciprocal(rz, zz)
                sm = pc0.tile([128, K1, 64], F32, tag="sm")
                nc.vector.tensor_mul(sm, esb.rearrange("p (k c) -> p k c", c=64),
                                     rz.rearrange("p (k u) -> p k u", u=1)
                                       .to_broadcast([128, K1, 64]))
                # attention
                pr = pc0.tile([128, K1, 32], F32, tag="pr")
                att = pc0.tile([128, 67], F32, tag="att")
                nc.gpsimd.tensor_mul(pr, sm[:, :, 0:32], rec0[:, :, 40:72])
                nc.vector.tensor_reduce(
                    att[:, 0:32], pr.rearrange("p k c -> p c k"),
                    axis=mybir.AxisListType.X, op=mybir.AluOpType.add)
                nc.vector.tensor_mul(pr, sm[:, :, 32:64],
                                     r0.rearrange("p (k c) -> p k c", c=32))
                nc.vector.tensor_reduce(
                    att[:, 32:64], pr.rearrange("p k c -> p c k"),
                    axis=mybir.AxisListType.X, op=mybir.AluOpType.add)
                nc.vector.tensor_copy(att[:, 64:66], gg[:, 4:6])
                nc.vector.memset(att[:, 66:67], 1.0)

                # x1 chunk = relu(w_f0^T att)
                psa = pc0_ps.tile([67, 128], F32, tag="pssm")
                tr(psa, att)
                attT = pc0.tile([67, 128], F32, tag="attT")
                nc.scalar.activation(attT, psa, AF.Copy)
                psx = pc0_ps.tile([64, 128], F32, tag="pssm")
                nc.tensor.matmul(psx, wsb["w_f0"], attT, start=True, stop=True)
                nc.scalar.activation(x1T[0:64, 128 * t:128 * (t + 1)], psx, AF.Relu)

            for c in range(0, R, min(N, 2048)):
                ce = min(c + min(N, 2048), R)
                nc.sync.dma_start(out=x1T[64:65, c:ce],
                                  in_=wt["ones_n"].ap()[:, 0:ce - c])
            # P1T + table1 records
            p1t = pc0.tile([128, R], F32, tag="p1t")
            for c in range(0, R, 512):
                ce = min(c + 512, R)
                ps = pc0_ps.tile([128, 512], F32, tag="pssm")
                nc.tensor.matmul(ps[:, 0:ce - c], wsb["w_p1"], x1T[:, c:ce],
                                 start=True, stop=True)
                nc.scalar.activation(p1t[:, c:ce], ps[:, 0:ce - c], AF.Copy)
            for t in range(T):
                ps = pc0_ps.tile([128, 192], F32, tag="pssm")
                sl = slice(128 * t, 128 * (t + 1))
                tr(ps[:, 0:64], x1T[0:64, sl])
                tr(ps[:, 64:192], p1t[:, sl])
                tbt1 = pc0.tile([128, 192], F32, tag="tbt1")
                nc.vector.tensor_copy(tbt1, ps)
                nc.sync.dma_start(out=table1_own.ap()[sl, 64:256], in_=tbt1)

        # ---- AllGather ----
        if no_cc:
            nc.sync.dma_start(out=table1.ap()[0:R, :], in_=table1_own.ap())
        elif CPB > 1:
            nc.gpsimd.collective_compute(
                "AllGather", mybir.AluOpType.bypass,
                replica_groups=groups,
                ins=[table1_own.ap()], outs=[table1.ap()])
        else:
            nc.sync.dma_start(out=table1.ap(), in_=table1_own.ap())

        # ================= PHASE D: stage 1 + final =================
        with tc.tile_pool(name="pd", bufs=2) as pd, \
             tc.tile_pool(name="pd_ps", bufs=2, space="PSUM") as pd_ps:
            if skip_stage1:
                zout = pd.tile([128, 128], F32, tag="zout")
                nc.vector.memset(zout, 0.0)
                for t in range(T):
                    nc.sync.dma_start(out=out.ap()[128 * t:128 * (t + 1), :],
                                      in_=zout)
            for t in range(0 if skip_stage1 else T):
                rec1 = pd.tile([128, K1, REC1], F32, tag="rec1")
                if gather_mode == "swdge":
                    nc.gpsimd.dma_gather(
                        out_ap=rec1, in_ap=table1.ap(),
                        idxs_ap=widx[:, S17 * t:S17 * (t + 1)],
                        num_idxs=K1 * 128, num_idxs_reg=K1 * 128,
                        elem_size=REC1, single_packet=False)
                elif gather_mode == "fake":
                    nc.sync.dma_start(
                        out=rec1,
                        in_=table1.ap()[0:K1 * 128, :]
                            .rearrange("(k p) c -> p k c", p=128))
                else:
                    nc.vector.memset(rec1, 0.0)

                r1 = pd.tile([128, K1 * 64], F32, tag="r1")
                nc.vector.tensor_add(
                    r1.rearrange("p (k c) -> p k c", c=64), rec1[:, :, 0:64],
                    S1_keep[:, 64 * t:64 * (t + 1)].rearrange("p (u c) -> p u c", u=1).to_broadcast([128, K1, 64]))
                nc.vector.tensor_scalar_max(r1, r1, 0.0)

                # r1T2 [128, 9*128]: 2-neighbor-packed transposes of r1
                r1T2 = pd.tile([128, 9 * 128], F32, tag="r1T2")
                for pg in range(3):
                    js = range(4 * pg, min(4 * pg + 4, 9))
                    psk = pd_ps.tile([128, 512], F32, tag="pst1", bufs=4)
                    for j in js:
                        o = 128 * (j - 4 * pg)
                        if j < 8:
                            tr(psk[:, o:o + 128], r1[:, 128 * j:128 * (j + 1)])
                        else:
                            tr(psk[0:64, o:o + 128], r1[:, 1024:1088])
                    if js.stop == 9:
                        nc.scalar.activation(r1T2[0:64, 1024:1152],
                                             psk[0:64, 0:128], AF.Copy)
                    else:
                        nc.scalar.activation(r1T2[:, 512 * pg:512 * (pg + 1)],
                                             psk, AF.Copy)

                # srT [128, K1*128]: per-neighbor w_sr1 products
                srT = pd.tile([128, K1 * 128], F32, tag="sr1T")
                for pg in range(5):
                    ks = range(4 * pg, min(4 * pg + 4, K1))
                    psk = pd_ps.tile([128, 512], F32, tag="pst1", bufs=4)
                    for k in ks:
                        o = 128 * (k - 4 * pg)
                        if k < 16:
                            wname = "w_sr1o" if k % 2 else "w_sr1e"
                            nc.tensor.matmul(
                                psk[:, o:o + 128], wsb[wname],
                                r1T2[:, 128 * (k // 2):128 * (k // 2) + 128],
                                start=True, stop=True)
                        else:
                            nc.tensor.matmul(
                                psk[:, o:o + 128], wsb["w_sr1"],
                                r1T2[0:64, 1024:1152],
                                start=True, stop=True)
                    w = 128 * len(ks)
                    nc.scalar.activation(srT[:, 512 * pg:512 * pg + w],
                                         psk[:, 0:w], AF.Copy)

                sl = pd.tile([128, K1 * 128], F32, tag="slsb1")
                for pg in range(5):
                    ks = range(4 * pg, min(4 * pg + 4, K1))
                    psk = pd_ps.tile([128, 512], F32, tag="pst1", bufs=4)
                    for k in ks:
                        o = 128 * (k - 4 * pg)
                        tr(psk[:, o:o + 128], srT[:, 128 * k:128 * (k + 1)])
                    w = 128 * len(ks)
                    nc.scalar.activation(sl[:, 512 * pg:512 * pg + w],
                                         psk[:, 0:w], AF.Copy)
                nc.vector.tensor_add(sl.rearrange("p (k c) -> p k c", c=128),
                                     sl.rearrange("p (k c) -> p k c", c=128),
                                     rec1[:, :, 128:256])
                esb = pd.tile([128, K1 * 128], F32, tag="esb1", bufs=1)
                nc.scalar.activation(esb, sl, AF.Exp)
                zz = pd.tile([128, K1], F32, tag="zz1")
                nc.vector.tensor_reduce(zz, esb.rearrange("p (k c) -> p k c", c=128),
                                        axis=mybir.AxisListType.X,
                                        op=mybir.AluOpType.add)
                rz = pd.tile([128, K1], F32, tag="rz1")
                nc.vector.reciprocal(rz, zz)
                sm = pd.tile([128, K1, 128], F32, tag="sm1", bufs=1)
                nc.vector.tensor_mul(sm, esb.rearrange("p (k c) -> p k c", c=128),
                                     rz.rearrange("p (k u) -> p k u", u=1)
                                       .to_broadcast([128, K1, 128]))
                pr = pd.tile([128, K1, 64], F32, tag="pr1")
                att = pd.tile([128, 128], F32, tag="att1")
                nc.gpsimd.tensor_mul(pr, sm[:, :, 0:64], rec1[:, :, 64:128])
                nc.vector.tensor_reduce(
                    att[:, 0:64], pr.rearrange("p k c -> p c k"),
                    axis=mybir.AxisListType.X, op=mybir.AluOpType.add)
                nc.vector.tensor_mul(pr, sm[:, :, 64:128],
                                     r1.rearrange("p (k c) -> p k c", c=64))
                nc.vector.tensor_reduce(
                    att[:, 64:128], pr.rearrange("p k c -> p c k"),
                    axis=mybir.AxisListType.X, op=mybir.AluOpType.add)

                # x2 = relu(Wf1a^T att1 + Wf1b^T [ggf; 1] + bf1)
                psa = pd_ps.tile([128, 128], F32, tag="pssm1")
                tr(psa, att)
                attT = pd.tile([128, 128], F32, tag="attT1")
                nc.scalar.activation(attT, psa, AF.Copy)
                gco = pd.tile([128, 3], F32, tag="gco")
                nc.vector.tensor_copy(gco[:, 0:2], ggf_keep[:, 2 * t:2 * t + 2])
                nc.vector.memset(gco[:, 2:3], 1.0)
                psg = pd_ps.tile([3, 128], F32, tag="pssm1")
                tr(psg, gco)
                gcoT = pd.tile([3, 128], F32, tag="gcoT")
                nc.scalar.activation(gcoT, psg, AF.Copy)
                psx = pd_ps.tile([128, 128], F32, tag="pssm1")
                nc.tensor.matmul(psx, wsb["w_f1a"], attT, start=True, stop=False)
                nc.tensor.matmul(psx, wsb["w_f1b"], gcoT, start=False, stop=True)
                x2T = pd.tile([128, 128], F32, tag="x2T")
                nc.scalar.activation(x2T, psx, AF.Relu)

                # x3 = relu(W1^T x2 + b1); out = leaky(x3 + y, 0.2)
                pso = pd_ps.tile([128, 128], F32, tag="pssm1")
                nc.tensor.matmul(pso, wsb["w_out"], x2T, start=True, stop=True)
                x3T = pd.tile([128, 128], F32, tag="x3T")
                nc.scalar.activation(x3T, pso, AF.Relu, bias=wsb["b_out"][:, 0:1])
                nc.vector.tensor_add(x3T, x3T, y_resT[:, 128 * t:128 * (t + 1)])
                pos = pd.tile([128, 128], F32, tag="pos")
                nc.vector.tensor_scalar_max(pos, x3T, 0.0)
                nc.vector.tensor_scalar(x3T, x3T, 0.0, 0.2,
                                        op0=mybir.AluOpType.min,
                                        op1=mybir.AluOpType.mult)
                nc.vector.tensor_add(x3T, x3T, pos)

                psf = pd_ps.tile([128, 128], F32, tag="pssm1")
                tr(psf, x3T)
                orow = pd.tile([128, 128], F32, tag="orow")
                nc.scalar.activation(orow, psf, AF.Copy)
                nc.sync.dma_start(out=out.ap()[128 * t:128 * (t + 1), :], in_=orow)

        pp.release()

    nc.compile()
    return nc


_CACHE = {}


def _get_module(N, CPB, n_cores):
    key = (N, CPB, n_cores)
    if key not in _CACHE:
        _CACHE[key] = build_module(N, CPB, n_cores)
    return _CACHE[key]


def kernel(**inputs):
    pc = np.asarray(inputs["pc"])
    feats = np.asarray(inputs["feats"])
    Bv, N, _ = pc.shape
    n_cores = 8
    CPB = n_cores // Bv
    R = N // CPB
    nc = _get_module(N, CPB, n_cores)
    wts = host_weights(inputs, N)
    in_maps = []
    for c in range(n_cores):
        b = c // CPB
        s = (c % CPB) * R
        m = {"pc_b": np.ascontiguousarray(pc[b]),
             "feats_b": np.ascontiguousarray(feats[b]),
             "pc_own": np.ascontiguousarray(pc[b, s:s + R]),
             "feats_own": np.ascontiguousarray(feats[b, s:s + R])}
        m.update(wts)
        in_maps.append(m)
    res = run_bass_kernel_spmd(nc, in_maps, list(range(n_cores)))
    out = np.empty((Bv, N, 128), np.float32)
    for c in range(n_cores):
        b = c // CPB
        s = (c % CPB) * R
        out[b, s:s + R] = res.results[c]["out"]
    return out

